# revision 12
# baseline (speedup 1.0000x reference)
"""Trainium2 Bass kernel for nn_MoELayer (dense MoE with top-k routing).

Strategy (8 NeuronCores, SPMD):
  - Expert parallelism for the E=8 routed experts: core c owns expert c's
    MLP weights and computes w_c[token] * MLP_c(x) for ALL tokens, where
    w_c is the token's softmax gate score masked to its top-k experts
    (zero if expert c not selected).
  - Shared experts are split along the hidden dimension H: core c computes
    the H-slice [c*512,(c+1)*512) of both shared experts for all tokens,
    scaled by the shared gate scores.
  - Gate scores + softmax + top-k masking are computed on-device (fp32).
  - All per-core partial outputs live in a [O, B] feature-major DRAM
    accumulator; a ReduceScatter(add) combines them across cores, each
    core returning its [O/8, B] slice. Host concatenates + transposes.
  - Matmuls run as float32r (FP32 inputs truncated to FP22 in the PE
    array) at full 78.6 TF/s/core throughput; gate matmuls run true fp32.
"""

from contextlib import ExitStack

import numpy as np

import concourse.bass as bass
import concourse.mybir as mybir
from concourse.tile import TileContext
from concourse.bass_utils import run_bass_kernel_spmd
from concourse.masks import make_identity

# ---------------------------------------------------------------- dims
B, D, H, O = 8192, 1024, 4096, 1024
E, S = 8, 2
ES = E + S            # gate columns
NC = 8                # cores
TOPK = 2
HH = H // 2           # routed-expert H half (SBUF capacity)
HS = H // NC          # shared-expert H slice per core
CH = 512              # token chunk (matmul moving dim)
OP = O // 128         # output 128-row tiles

f32 = mybir.dt.float32
f32r = mybir.dt.float32r

# ------------------------------------------------- walrus sync-wait workaround
# This walrus build rejects any instruction carrying more than one semaphore
# wait ("Too many sync wait commands" in setupSyncWait). Tile's semaphore
# pass freely attaches several waits to one instruction. Post-process the
# serialized BIR: hoist all-but-one wait of each instruction onto standalone
# same-engine NoOps inserted immediately before it (same-engine program order
# preserves semantics exactly).
import json as _json


def _split_multi_waits(nc):
    d = _json.loads(mybir.module_to_json_string(nc.m))
    nsplit = 0
    for fn in d["functions"]:
        for bb in fn["blocks"]:
            out = []
            for inst in bb["instructions"]:
                si = inst.get("sync_info")
                waits = (si or {}).get("on_wait") or []
                if len(waits) > 1:
                    for j, w in enumerate(waits[:-1]):
                        nop = {
                            "engine": inst["engine"],
                            "ins": [],
                            "outs": [],
                            "name": f"{inst['name']}-w{j}",
                            "opcode": "NoOp",
                            "sync_info": {"on_wait": [w], "on_update": []},
                        }
                        if "debug" in inst:
                            nop["debug"] = inst["debug"]
                        out.append(nop)
                        nsplit += 1
                    si["on_wait"] = [waits[-1]]
                out.append(inst)
            bb["instructions"] = out
    nc.m = mybir.module_from_json_string(_json.dumps(d))
    return nsplit


# ---------------------------------------------------------------- builder
def _bias_col(nc, dst, src_1d):
    """DMA a length-128 1-D DRAM slice into a [128, 1] SBUF column."""
    nc.sync.dma_start(out=dst, in_=src_1d.rearrange("(p o) -> p o", o=1))



def _bcast_row(nc, psum_pool, out_pool, ones_col, row_ap, n, tag):
    """Broadcast a [1, n] SBUF row to a [128, n] tile: ones[1,128].T @ row."""
    ps = psum_pool.tile([128, n], f32, tag=tag + "_ps")
    nc.tensor.matmul(ps[:], lhsT=ones_col[:], rhs=row_ap)
    t = out_pool.tile([128, n], f32, tag=tag)
    nc.vector.tensor_copy(t[:], ps[:])
    return t


def build(nbatch: int) -> bass.Bass:
    assert nbatch % CH == 0
    nch = nbatch // CH

    nc = bass.Bass()
    xTf = nc.declare_dram_parameter("xTf", [D, nbatch], f32, isOutput=False)
    xTr = nc.declare_dram_parameter("xTr", [D, nbatch], f32r, isOutput=False)
    w1e = nc.declare_dram_parameter("w1e", [D, H], f32r, isOutput=False)
    w2e = nc.declare_dram_parameter("w2e", [H, O], f32r, isOutput=False)
    w1s = nc.declare_dram_parameter("w1s", [S, D, HS], f32r, isOutput=False)
    w2s = nc.declare_dram_parameter("w2s", [S, HS, O], f32r, isOutput=False)
    wg = nc.declare_dram_parameter("wg", [D, ES], f32, isOutput=False)
    bg = nc.declare_dram_parameter("bg", [ES, 1], f32, isOutput=False)
    b1 = nc.declare_dram_parameter("b1", [H], f32, isOutput=False)
    b2 = nc.declare_dram_parameter("b2", [O], f32, isOutput=False)
    bs1 = nc.declare_dram_parameter("bs1", [S, HS], f32, isOutput=False)
    bs2 = nc.declare_dram_parameter("bs2", [S, O], f32, isOutput=False)  # /NC on host
    sel = nc.declare_dram_parameter("sel", [1, E], f32, isOutput=False)
    y = nc.declare_dram_parameter("y", [O // NC, nbatch], f32, isOutput=True)

    acc = nc.dram_tensor("acc", [O, nbatch], f32)
    rs = nc.dram_tensor("rs", [O // NC, nbatch], f32)
    g0d = nc.dram_tensor("g0d", [1, nbatch], f32)
    g1d = nc.dram_tensor("g1d", [1, nbatch], f32)
    wed = nc.dram_tensor("wed", [1, nbatch], f32)

    Relu = mybir.ActivationFunctionType.Relu
    Ident = mybir.ActivationFunctionType.Identity
    Exp = mybir.ActivationFunctionType.Exp
    AX = mybir.AxisListType.X

    with TileContext(nc) as tc:
        # ---------------- phase 0: gate scores, softmax, top-k mask ----------
        with ExitStack() as gx:
            gconst = gx.enter_context(tc.tile_pool(name="gconst", bufs=1))
            gp = gx.enter_context(tc.tile_pool(name="gp", bufs=3))
            gxp = gx.enter_context(tc.tile_pool(name="gxp", bufs=3))
            gps = gx.enter_context(tc.tile_pool(name="gps", bufs=2, space="PSUM"))
            gps2 = gx.enter_context(tc.tile_pool(name="gps2", bufs=2, space="PSUM"))

            ident = gconst.tile([128, 128], f32, tag="ident")
            make_identity(nc, ident)
            wg_sb = gconst.tile([128, 8 * ES], f32, tag="wg_sb")
            for k in range(8):
                nc.sync.dma_start(
                    out=wg_sb[:, k * ES : (k + 1) * ES],
                    in_=wg[k * 128 : (k + 1) * 128, :],
                )
            bg_sb = gconst.tile([ES, 1], f32, tag="bg_sb")
            nc.sync.dma_start(out=bg_sb[:], in_=bg[:])
            sel_st = gconst.tile([1, E], f32, tag="sel_st")
            nc.sync.dma_start(out=sel_st[:], in_=sel[:])
            ones_g = gconst.tile([1, 128], f32, tag="ones_g")
            nc.vector.memset(ones_g[:], 1.0)
            selb = _bcast_row(nc, gps2, gconst, ones_g, sel_st[:], E, "selb")

            for c in range(nch):
                csl = slice(c * CH, (c + 1) * CH)
                xc = []
                for k in range(8):
                    t = gxp.tile([128, CH], f32, tag=f"gx{k}")
                    nc.sync.dma_start(
                        out=t[:], in_=xTf[k * 128 : (k + 1) * 128, csl]
                    )
                    xc.append(t)
                psg = gps.tile([ES, CH], f32, tag="psg")
                for k in range(8):
                    nc.tensor.matmul(
                        psg[:],
                        lhsT=wg_sb[:, k * ES : (k + 1) * ES],
                        rhs=xc[k][:],
                        start=(k == 0),
                        stop=(k == 7),
                    )
                gts = gp.tile([ES, CH], f32, tag="gts")
                nc.scalar.activation(gts[:], psg[:], Ident, bias=bg_sb[:])

                for blk in range(CH // 128):
                    bsl = slice(blk * 128, (blk + 1) * 128)
                    pst = gps2.tile([128, 128], f32, tag="pst")
                    # [ES, 128] -> [128, ES]
                    nc.tensor.matmul(
                        pst[:, :ES],
                        lhsT=gts[:, bsl],
                        rhs=ident[:ES, :ES],
                        is_transpose=True,
                    )
                    gtm = gp.tile([128, ES], f32, tag="gtm")
                    nc.vector.tensor_copy(gtm[:], pst[:, :ES])
                    mx = gp.tile([128, 1], f32, tag="mx")
                    nc.vector.reduce_max(mx[:], gtm[:], axis=AX)
                    nmx = gp.tile([128, 1], f32, tag="nmx")
                    nc.vector.tensor_scalar_mul(nmx[:], mx[:], -1.0)
                    ex = gp.tile([128, ES], f32, tag="ex")
                    nc.scalar.activation(ex[:], gtm[:], Exp, bias=nmx[:])
                    sm = gp.tile([128, 1], f32, tag="sm")
                    nc.vector.reduce_sum(sm[:], ex[:], axis=AX)
                    rc = gp.tile([128, 1], f32, tag="rc")
                    nc.vector.reciprocal(rc[:], sm[:])
                    pr = gp.tile([128, ES], f32, tag="pr")
                    nc.vector.tensor_scalar_mul(pr[:], ex[:], rc[:])
                    # top-k mask over routed columns
                    m8 = gp.tile([128, 8], f32, tag="m8")
                    nc.vector.max(m8[:], pr[:, S:])
                    nc.vector.memset(m8[:, TOPK:], -1.0)
                    rep = gp.tile([128, 8], f32, tag="rep")
                    nc.vector.match_replace(
                        rep[:], in_to_replace=m8[:], in_values=pr[:, S:], imm_value=0.0
                    )
                    wr = gp.tile([128, ES + 1], f32, tag="wr")
                    nc.vector.tensor_copy(wr[:, :S], pr[:, :S])
                    nc.vector.tensor_sub(wr[:, S : ES], pr[:, S:], rep[:])
                    # this core's expert gate: dot(masked routed, one-hot)
                    seld = gp.tile([128, E], f32, tag="seld")
                    nc.vector.tensor_mul(seld[:], wr[:, S:ES], selb[:])
                    nc.vector.reduce_sum(wr[:, ES : ES + 1], seld[:], axis=AX)
                    # [128, ES+1] -> [ES+1, 128]
                    pst2 = gps2.tile([128, 128], f32, tag="pst2")
                    nc.tensor.matmul(
                        pst2[: ES + 1, :], lhsT=wr[:], rhs=ident[:, :], is_transpose=True
                    )
                    wtb = gp.tile([ES + 1, 128], f32, tag="wtb")
                    nc.vector.tensor_copy(wtb[:], pst2[: ES + 1, :])
                    bdst = slice(c * CH + blk * 128, c * CH + (blk + 1) * 128)
                    nc.sync.dma_start(out=g0d[0:1, bdst], in_=wtb[0:1, :])
                    nc.sync.dma_start(out=g1d[0:1, bdst], in_=wtb[1:2, :])
                    nc.sync.dma_start(out=wed[0:1, bdst], in_=wtb[ES : ES + 1, :])

        # ---------------- phase 1+2: routed expert, H halves -----------------
        for hf in range(2):
            with ExitStack() as rx:
                wp = rx.enter_context(tc.tile_pool(name=f"wr{hf}", bufs=1))
                xp = rx.enter_context(tc.tile_pool(name=f"xr{hf}", bufs=2))
                hp = rx.enter_context(tc.tile_pool(name=f"hr{hf}", bufs=1))
                op_ = rx.enter_context(tc.tile_pool(name=f"or{hf}", bufs=2))
                bp = rx.enter_context(tc.tile_pool(name=f"br{hf}", bufs=2))
                pp1 = rx.enter_context(tc.tile_pool(name=f"p1r{hf}", bufs=2, space="PSUM"))
                pp2 = rx.enter_context(tc.tile_pool(name=f"p2r{hf}", bufs=2, space="PSUM"))

                w1t = []
                for k in range(8):
                    t = wp.tile([128, HH], f32r, tag=f"w1t{k}")
                    nc.sync.dma_start(
                        out=t[:], in_=w1e[k * 128 : (k + 1) * 128, hf * HH : (hf + 1) * HH]
                    )
                    w1t.append(t)
                w2t = []
                for kh in range(HH // 128):
                    t = wp.tile([128, O], f32r, tag=f"w2t{kh}")
                    nc.sync.dma_start(
                        out=t[:],
                        in_=w2e[hf * HH + kh * 128 : hf * HH + (kh + 1) * 128, :],
                    )
                    w2t.append(t)
                b1_sb = wp.tile([128, HH // 128], f32, tag="b1_sb")
                for ht in range(HH // 128):
                    _bias_col(
                        nc,
                        b1_sb[:, ht : ht + 1],
                        b1[hf * HH + ht * 128 : hf * HH + (ht + 1) * 128],
                    )
                ones_r = wp.tile([1, 128], f32, tag="ones_r")
                nc.vector.memset(ones_r[:], 1.0)
                b2_sb = wp.tile([128, OP], f32, tag="b2_sb")
                if hf == 0:
                    for o in range(OP):
                        _bias_col(nc, b2_sb[:, o : o + 1], b2[o * 128 : (o + 1) * 128])

                for c in range(nch):
                    csl = slice(c * CH, (c + 1) * CH)
                    xc = []
                    for k in range(8):
                        t = xp.tile([128, CH], f32r, tag=f"x{k}")
                        nc.sync.dma_start(
                            out=t[:], in_=xTr[k * 128 : (k + 1) * 128, csl]
                        )
                        xc.append(t)
                    wst = bp.tile([1, CH], f32, tag="wst")
                    nc.sync.dma_start(out=wst[:], in_=wed[0:1, csl])
                    wb = _bcast_row(nc, pp2, bp, ones_r, wst[:], CH, "wb")

                    hts = []
                    for ht in range(HH // 128):
                        ps = pp1.tile([128, CH], f32, tag="ps1")
                        for k in range(8):
                            nc.tensor.matmul(
                                ps[:],
                                lhsT=w1t[k][:, ht * 128 : (ht + 1) * 128],
                                rhs=xc[k][:],
                                start=(k == 0),
                                stop=(k == 7),
                            )
                        hsb = hp.tile([128, CH], f32r, tag=f"h{ht}")
                        nc.scalar.activation(
                            hsb[:], ps[:], Relu, bias=b1_sb[:, ht : ht + 1]
                        )
                        hts.append(hsb)

                    for o in range(OP):
                        ps2 = pp2.tile([128, CH], f32, tag="ps2")
                        for kh in range(HH // 128):
                            nc.tensor.matmul(
                                ps2[:],
                                lhsT=w2t[kh][:, o * 128 : (o + 1) * 128],
                                rhs=hts[kh][:],
                                start=(kh == 0),
                                stop=(kh == HH // 128 - 1),
                            )
                        ot = op_.tile([128, CH], f32, tag="ot")
                        if hf == 0:
                            nc.vector.tensor_scalar_add(ot[:], ps2[:], b2_sb[:, o : o + 1])
                            nc.vector.tensor_mul(ot[:], ot[:], wb[:])
                            nc.sync.dma_start(
                                out=acc[o * 128 : (o + 1) * 128, csl], in_=ot[:]
                            )
                        else:
                            nc.vector.tensor_mul(ot[:], ps2[:], wb[:])
                            nc.gpsimd.dma_start(
                                out=acc[o * 128 : (o + 1) * 128, csl],
                                in_=ot[:],
                                accum_op=mybir.AluOpType.add,
                            )

        # ---------------- phase 3: shared experts (H-sliced) -----------------
        with ExitStack() as sx:
            wp = sx.enter_context(tc.tile_pool(name="ws", bufs=1))
            xp = sx.enter_context(tc.tile_pool(name="xs", bufs=2))
            hp = sx.enter_context(tc.tile_pool(name="hs", bufs=1))
            op_ = sx.enter_context(tc.tile_pool(name="os", bufs=4))
            bp = sx.enter_context(tc.tile_pool(name="bs", bufs=2))
            pp1 = sx.enter_context(tc.tile_pool(name="p1s", bufs=2, space="PSUM"))
            pp2 = sx.enter_context(tc.tile_pool(name="p2s", bufs=2, space="PSUM"))

            w1st, w2st = {}, {}
            for s in range(S):
                for k in range(8):
                    t = wp.tile([128, HS], f32r, tag=f"w1s{s}_{k}")
                    nc.sync.dma_start(out=t[:], in_=w1s[s, k * 128 : (k + 1) * 128, :])
                    w1st[s, k] = t
                for kh in range(HS // 128):
                    t = wp.tile([128, O], f32r, tag=f"w2s{s}_{kh}")
                    nc.sync.dma_start(
                        out=t[:], in_=w2s[s, kh * 128 : (kh + 1) * 128, :]
                    )
                    w2st[s, kh] = t
            bs1_sb = wp.tile([128, S * (HS // 128)], f32, tag="bs1_sb")
            for s in range(S):
                for ht in range(HS // 128):
                    _bias_col(
                        nc,
                        bs1_sb[:, s * (HS // 128) + ht : s * (HS // 128) + ht + 1],
                        bs1[s, ht * 128 : (ht + 1) * 128],
                    )
            ones_s = wp.tile([1, 128], f32, tag="ones_s")
            nc.vector.memset(ones_s[:], 1.0)
            bs2_sb = wp.tile([128, S * OP], f32, tag="bs2_sb")
            for s in range(S):
                for o in range(OP):
                    _bias_col(
                        nc,
                        bs2_sb[:, s * OP + o : s * OP + o + 1],
                        bs2[s, o * 128 : (o + 1) * 128],
                    )

            for c in range(nch):
                csl = slice(c * CH, (c + 1) * CH)
                xc = []
                for k in range(8):
                    t = xp.tile([128, CH], f32r, tag=f"xs{k}")
                    nc.sync.dma_start(out=t[:], in_=xTr[k * 128 : (k + 1) * 128, csl])
                    xc.append(t)
                gb = []
                for s, gsrc in ((0, g0d), (1, g1d)):
                    st = bp.tile([1, CH], f32, tag=f"gst{s}")
                    nc.sync.dma_start(out=st[:], in_=gsrc[0:1, csl])
                    gb.append(_bcast_row(nc, pp2, bp, ones_s, st[:], CH, f"gb{s}"))

                hts = {}
                for s in range(S):
                    for ht in range(HS // 128):
                        ps = pp1.tile([128, CH], f32, tag="ps1s")
                        for k in range(8):
                            nc.tensor.matmul(
                                ps[:],
                                lhsT=w1st[s, k][:, ht * 128 : (ht + 1) * 128],
                                rhs=xc[k][:],
                                start=(k == 0),
                                stop=(k == 7),
                            )
                        hsb = hp.tile([128, CH], f32r, tag=f"hs{s}_{ht}")
                        nc.scalar.activation(
                            hsb[:],
                            ps[:],
                            Relu,
                            bias=bs1_sb[:, s * (HS // 128) + ht : s * (HS // 128) + ht + 1],
                        )
                        hts[s, ht] = hsb

                for o in range(OP):
                    acc_t = op_.tile([128, CH], f32, tag="acct")
                    for s in range(S):
                        ps2 = pp2.tile([128, CH], f32, tag="ps2s")
                        for kh in range(HS // 128):
                            nc.tensor.matmul(
                                ps2[:],
                                lhsT=w2st[s, kh][:, o * 128 : (o + 1) * 128],
                                rhs=hts[s, kh][:],
                                start=(kh == 0),
                                stop=(kh == HS // 128 - 1),
                            )
                        tmp = op_.tile([128, CH], f32, tag="tmps")
                        nc.vector.tensor_scalar_add(
                            tmp[:], ps2[:], bs2_sb[:, s * OP + o : s * OP + o + 1]
                        )
                        if s == 0:
                            nc.vector.tensor_mul(acc_t[:], tmp[:], gb[s][:])
                        else:
                            nc.vector.tensor_mul(tmp[:], tmp[:], gb[s][:])
                            nc.vector.tensor_add(acc_t[:], acc_t[:], tmp[:])
                    nc.gpsimd.dma_start(
                        out=acc[o * 128 : (o + 1) * 128, csl],
                        in_=acc_t[:],
                        accum_op=mybir.AluOpType.add,
                    )

        # ---------------- phase 4: combine across cores ----------------------
        nc.gpsimd.collective_compute(
            "ReduceScatter",
            mybir.AluOpType.add,
            replica_groups=[list(range(NC))],
            ins=[acc[:]],
            outs=[rs[:]],
        )
        nc.sync.dma_start(out=y[:], in_=rs[:])

    _split_multi_waits(nc)
    return nc


# ---------------------------------------------------------------- host side
_cache = {}


def _get_nc(nbatch):
    if nbatch not in _cache:
        _cache[nbatch] = build(nbatch)
    return _cache[nbatch]


def _make_in_maps(x, W1, b1, W2, b2, Ws1, bs1, Ws2, bs2, Wg, bg):
    x = np.asarray(x, np.float32)
    xT = np.ascontiguousarray(x.T)
    W1 = np.asarray(W1, np.float32)
    W2 = np.asarray(W2, np.float32)
    Ws1 = np.asarray(Ws1, np.float32)
    Ws2 = np.asarray(Ws2, np.float32)
    Wg = np.asarray(Wg, np.float32)
    bg = np.asarray(bg, np.float32)
    b1 = np.asarray(b1, np.float32)
    b2 = np.asarray(b2, np.float32)
    bs1 = np.asarray(bs1, np.float32)
    bs2 = np.asarray(bs2, np.float32)

    in_maps = []
    for c in range(NC):
        sel = np.zeros((1, E), np.float32)
        sel[0, c] = 1.0
        in_maps.append(
            {
                "xTf": xT,
                "xTr": xT,
                "w1e": np.ascontiguousarray(W1[c]),
                "w2e": np.ascontiguousarray(W2[c]),
                "w1s": np.ascontiguousarray(Ws1[:, :, c * HS : (c + 1) * HS]),
                "w2s": np.ascontiguousarray(Ws2[:, c * HS : (c + 1) * HS, :]),
                "wg": Wg,
                "bg": bg.reshape(ES, 1),
                "b1": np.ascontiguousarray(b1[c]),
                "b2": np.ascontiguousarray(b2[c]),
                "bs1": np.ascontiguousarray(bs1[:, c * HS : (c + 1) * HS]),
                "bs2": bs2 / float(NC),
                "sel": sel,
            }
        )
    return in_maps


def kernel(x, W1, b1, W2, b2, Ws1, bs1, Ws2, bs2, Wg, bg, k):
    assert int(k) == TOPK
    nbatch = np.asarray(x).shape[0]
    nc = _get_nc(nbatch)
    in_maps = _make_in_maps(x, W1, b1, W2, b2, Ws1, bs1, Ws2, bs2, Wg, bg)
    res = run_bass_kernel_spmd(nc, in_maps, list(range(NC)))
    outT = np.concatenate([res.results[c]["y"] for c in range(NC)], axis=0)
    return np.ascontiguousarray(outT.T)


def bench(inputs, iters=8):
    """Run the kernel via a non-donating jit: returns (output, min_ns).
    Pre-stages sharded device inputs so repeat timings exclude H2D."""
    import time

    import jax
    from jax.experimental.shard_map import shard_map
    from jax.sharding import Mesh, NamedSharding, PartitionSpec

    from concourse import bass2jax

    nbatch = np.asarray(inputs["x"]).shape[0]
    nc = _get_nc(nbatch)
    in_maps = _make_in_maps(**{k: v for k, v in inputs.items() if k != "k"})

    partition_name = nc.partition_id_tensor.name if nc.partition_id_tensor else None
    in_names, out_names, out_avals, zero_outs = [], [], [], []
    for alloc in nc.m.functions[0].allocations:
        if not isinstance(alloc, mybir.MemoryLocationSet):
            continue
        name = alloc.memorylocations[0].name
        if alloc.kind == "ExternalInput":
            if name != partition_name:
                in_names.append(name)
        elif alloc.kind == "ExternalOutput":
            shape = tuple(alloc.tensor_shape)
            dt_ = mybir.dt.np(alloc.dtype)
            out_names.append(name)
            out_avals.append(jax.core.ShapedArray(shape, dt_))
            zero_outs.append(np.zeros(shape, dt_))
    n_params = len(in_names)
    bind_names = list(in_names) + list(out_names)
    if partition_name is not None:
        bind_names.append(partition_name)

    def _body(*args):
        operands = list(args)
        if partition_name is not None:
            operands.append(bass2jax.partition_id_tensor())
        outs = bass2jax._bass_exec_p.bind(
            *operands,
            out_avals=tuple(out_avals),
            in_names=tuple(bind_names),
            out_names=tuple(out_names),
            lowering_input_output_aliases=(),
            sim_require_finite=True,
            sim_require_nnan=True,
            nc=nc,
        )
        return tuple(outs)

    devices = jax.devices()[:NC]
    mesh = Mesh(np.asarray(devices), ("core",))
    nin = n_params + len(out_names)
    fn = jax.jit(
        shard_map(
            _body,
            mesh=mesh,
            in_specs=(PartitionSpec("core"),) * nin,
            out_specs=(PartitionSpec("core"),) * len(out_names),
            check_rep=False,
        ),
        keep_unused=True,
    )
    sh = NamedSharding(mesh, PartitionSpec("core"))
    concat_in = [
        np.concatenate([np.asarray(in_maps[c][n]) for c in range(NC)], axis=0)
        for n in in_names
    ]
    concat_zeros = [
        np.zeros((NC * z.shape[0], *z.shape[1:]), z.dtype) for z in zero_outs
    ]
    args = [jax.device_put(a, sh) for a in concat_in + concat_zeros]
    jax.block_until_ready(args)
    jax.block_until_ready(fn(*args))  # compile + warm
    out_arrs = fn(*args)
    jax.block_until_ready(out_arrs)
    times = []
    for _ in range(iters):
        t0 = time.perf_counter()
        jax.block_until_ready(fn(*args))
        times.append(time.perf_counter() - t0)
    times.sort()
    print(f"bench times (s): min={times[0]:.4f} med={times[len(times)//2]:.4f} max={times[-1]:.4f}", flush=True)
    yc = np.asarray(out_arrs[out_names.index("y")])
    outT = yc.reshape(NC, O // NC, -1).reshape(O, -1)
    result = np.ascontiguousarray(outT.T)
    return result, times[0] * 1e9


# revision 13
# speedup vs baseline: 1.4997x; 1.4997x over previous
"""Trainium2 Bass kernel for nn_MoELayer (dense MoE with top-k routing).

Strategy (8 NeuronCores, SPMD):
  - Expert parallelism for the E=8 routed experts: core c owns expert c's
    MLP weights and computes w_c[token] * MLP_c(x) for ALL tokens, where
    w_c is the token's softmax gate score masked to its top-k experts
    (zero if expert c not selected).
  - Shared experts are split along the hidden dimension H: core c computes
    the H-slice [c*512,(c+1)*512) of both shared experts for all tokens,
    scaled by the shared gate scores.
  - Gate scores + softmax + top-k masking are computed on-device (fp32).
  - All per-core partial outputs live in a [O, B] feature-major DRAM
    accumulator; a ReduceScatter(add) combines them across cores, each
    core returning its [O/8, B] slice. Host concatenates + transposes.
  - Matmuls run as float32r (FP32 inputs truncated to FP22 in the PE
    array) at full 78.6 TF/s/core throughput; gate matmuls run true fp32.
"""

from contextlib import ExitStack

import numpy as np

import concourse.bass as bass
import concourse.mybir as mybir
from concourse.tile import TileContext
from concourse.masks import make_identity

# ---------------------------------------------------------------- dims
B, D, H, O = 8192, 1024, 4096, 1024
E, S = 8, 2
ES = E + S            # gate columns
NC = 8                # cores
TOPK = 2
HH = H // 2           # routed-expert H half (SBUF capacity)
HS = H // NC          # shared-expert H slice per core
CH = 512              # token chunk (matmul moving dim)
OP = O // 128         # output 128-row tiles

f32 = mybir.dt.float32
f32r = mybir.dt.float32r

# ------------------------------------------------- walrus sync-wait workaround
# This walrus build rejects any instruction carrying more than one semaphore
# wait ("Too many sync wait commands" in setupSyncWait). Tile's semaphore
# pass freely attaches several waits to one instruction. Post-process the
# serialized BIR: hoist all-but-one wait of each instruction onto standalone
# same-engine NoOps inserted immediately before it (same-engine program order
# preserves semantics exactly).
import json as _json


def _split_multi_waits(nc):
    d = _json.loads(mybir.module_to_json_string(nc.m))
    nsplit = 0
    for fn in d["functions"]:
        for bb in fn["blocks"]:
            out = []
            for inst in bb["instructions"]:
                si = inst.get("sync_info")
                waits = (si or {}).get("on_wait") or []
                if len(waits) > 1:
                    for j, w in enumerate(waits[:-1]):
                        nop = {
                            "engine": inst["engine"],
                            "ins": [],
                            "outs": [],
                            "name": f"{inst['name']}-w{j}",
                            "opcode": "NoOp",
                            "sync_info": {"on_wait": [w], "on_update": []},
                        }
                        if "debug" in inst:
                            nop["debug"] = inst["debug"]
                        out.append(nop)
                        nsplit += 1
                    si["on_wait"] = [waits[-1]]
                out.append(inst)
            bb["instructions"] = out
    nc.m = mybir.module_from_json_string(_json.dumps(d))
    return nsplit


# ---------------------------------------------------------------- builder
def _bias_col(nc, dst, src_1d):
    """DMA a length-128 1-D DRAM slice into a [128, 1] SBUF column."""
    nc.sync.dma_start(out=dst, in_=src_1d.rearrange("(p o) -> p o", o=1))



def _bcast_row(nc, psum_pool, out_pool, ones_col, row_ap, n, tag):
    """Broadcast a [1, n] SBUF row to a [128, n] tile: ones[1,128].T @ row."""
    ps = psum_pool.tile([128, n], f32, tag=tag + "_ps")
    nc.tensor.matmul(ps[:], lhsT=ones_col[:], rhs=row_ap)
    t = out_pool.tile([128, n], f32, tag=tag)
    nc.vector.tensor_copy(t[:], ps[:])
    return t


def build(nbatch: int) -> bass.Bass:
    assert nbatch % CH == 0
    nch = nbatch // CH

    nc = bass.Bass()
    xTf = nc.declare_dram_parameter("xTf", [D, nbatch], f32, isOutput=False)
    xTr = nc.declare_dram_parameter("xTr", [D, nbatch], f32r, isOutput=False)
    w1e = nc.declare_dram_parameter("w1e", [D, H], f32r, isOutput=False)
    w2e = nc.declare_dram_parameter("w2e", [H, O], f32r, isOutput=False)
    w1s = nc.declare_dram_parameter("w1s", [S, D, HS], f32r, isOutput=False)
    w2s = nc.declare_dram_parameter("w2s", [S, HS, O], f32r, isOutput=False)
    wg = nc.declare_dram_parameter("wg", [D, ES], f32, isOutput=False)
    bg = nc.declare_dram_parameter("bg", [ES, 1], f32, isOutput=False)
    b1 = nc.declare_dram_parameter("b1", [H], f32, isOutput=False)
    b2 = nc.declare_dram_parameter("b2", [O], f32, isOutput=False)
    bs1 = nc.declare_dram_parameter("bs1", [S, HS], f32, isOutput=False)
    bs2 = nc.declare_dram_parameter("bs2", [S, O], f32, isOutput=False)  # /NC on host
    sel = nc.declare_dram_parameter("sel", [1, E], f32, isOutput=False)
    y = nc.declare_dram_parameter("y", [O // NC, nbatch], f32, isOutput=True)

    acc = nc.dram_tensor("acc", [O, nbatch], f32)
    rs = nc.dram_tensor("rs", [O // NC, nbatch], f32)
    g0d = nc.dram_tensor("g0d", [1, nbatch], f32)
    g1d = nc.dram_tensor("g1d", [1, nbatch], f32)
    wed = nc.dram_tensor("wed", [1, nbatch], f32)

    Relu = mybir.ActivationFunctionType.Relu
    Ident = mybir.ActivationFunctionType.Identity
    Exp = mybir.ActivationFunctionType.Exp
    AX = mybir.AxisListType.X

    with TileContext(nc) as tc:
        # ---------------- phase 0: gate scores, softmax, top-k mask ----------
        with ExitStack() as gx:
            gconst = gx.enter_context(tc.tile_pool(name="gconst", bufs=1))
            gp = gx.enter_context(tc.tile_pool(name="gp", bufs=3))
            gxp = gx.enter_context(tc.tile_pool(name="gxp", bufs=3))
            gps = gx.enter_context(tc.tile_pool(name="gps", bufs=2, space="PSUM"))
            gps2 = gx.enter_context(tc.tile_pool(name="gps2", bufs=2, space="PSUM"))

            ident = gconst.tile([128, 128], f32, tag="ident")
            make_identity(nc, ident)
            wg_sb = gconst.tile([128, 8 * ES], f32, tag="wg_sb")
            for k in range(8):
                nc.sync.dma_start(
                    out=wg_sb[:, k * ES : (k + 1) * ES],
                    in_=wg[k * 128 : (k + 1) * 128, :],
                )
            bg_sb = gconst.tile([ES, 1], f32, tag="bg_sb")
            nc.sync.dma_start(out=bg_sb[:], in_=bg[:])
            sel_st = gconst.tile([1, E], f32, tag="sel_st")
            nc.sync.dma_start(out=sel_st[:], in_=sel[:])
            ones_g = gconst.tile([1, 128], f32, tag="ones_g")
            nc.vector.memset(ones_g[:], 1.0)
            selb = _bcast_row(nc, gps2, gconst, ones_g, sel_st[:], E, "selb")

            for c in range(nch):
                csl = slice(c * CH, (c + 1) * CH)
                xc = []
                for k in range(8):
                    t = gxp.tile([128, CH], f32, tag=f"gx{k}")
                    nc.sync.dma_start(
                        out=t[:], in_=xTf[k * 128 : (k + 1) * 128, csl]
                    )
                    xc.append(t)
                psg = gps.tile([ES, CH], f32, tag="psg")
                for k in range(8):
                    nc.tensor.matmul(
                        psg[:],
                        lhsT=wg_sb[:, k * ES : (k + 1) * ES],
                        rhs=xc[k][:],
                        start=(k == 0),
                        stop=(k == 7),
                    )
                gts = gp.tile([ES, CH], f32, tag="gts")
                nc.scalar.activation(gts[:], psg[:], Ident, bias=bg_sb[:])

                for blk in range(CH // 128):
                    bsl = slice(blk * 128, (blk + 1) * 128)
                    pst = gps2.tile([128, 128], f32, tag="pst")
                    # [ES, 128] -> [128, ES]
                    nc.tensor.matmul(
                        pst[:, :ES],
                        lhsT=gts[:, bsl],
                        rhs=ident[:ES, :ES],
                        is_transpose=True,
                    )
                    gtm = gp.tile([128, ES], f32, tag="gtm")
                    nc.vector.tensor_copy(gtm[:], pst[:, :ES])
                    mx = gp.tile([128, 1], f32, tag="mx")
                    nc.vector.reduce_max(mx[:], gtm[:], axis=AX)
                    nmx = gp.tile([128, 1], f32, tag="nmx")
                    nc.vector.tensor_scalar_mul(nmx[:], mx[:], -1.0)
                    ex = gp.tile([128, ES], f32, tag="ex")
                    nc.scalar.activation(ex[:], gtm[:], Exp, bias=nmx[:])
                    sm = gp.tile([128, 1], f32, tag="sm")
                    nc.vector.reduce_sum(sm[:], ex[:], axis=AX)
                    rc = gp.tile([128, 1], f32, tag="rc")
                    nc.vector.reciprocal(rc[:], sm[:])
                    pr = gp.tile([128, ES], f32, tag="pr")
                    nc.vector.tensor_scalar_mul(pr[:], ex[:], rc[:])
                    # top-k mask over routed columns
                    m8 = gp.tile([128, 8], f32, tag="m8")
                    nc.vector.max(m8[:], pr[:, S:])
                    nc.vector.memset(m8[:, TOPK:], -1.0)
                    rep = gp.tile([128, 8], f32, tag="rep")
                    nc.vector.match_replace(
                        rep[:], in_to_replace=m8[:], in_values=pr[:, S:], imm_value=0.0
                    )
                    wr = gp.tile([128, ES + 1], f32, tag="wr")
                    nc.vector.tensor_copy(wr[:, :S], pr[:, :S])
                    nc.vector.tensor_sub(wr[:, S : ES], pr[:, S:], rep[:])
                    # this core's expert gate: dot(masked routed, one-hot)
                    seld = gp.tile([128, E], f32, tag="seld")
                    nc.vector.tensor_mul(seld[:], wr[:, S:ES], selb[:])
                    nc.vector.reduce_sum(wr[:, ES : ES + 1], seld[:], axis=AX)
                    # [128, ES+1] -> [ES+1, 128]
                    pst2 = gps2.tile([128, 128], f32, tag="pst2")
                    nc.tensor.matmul(
                        pst2[: ES + 1, :], lhsT=wr[:], rhs=ident[:, :], is_transpose=True
                    )
                    wtb = gp.tile([ES + 1, 128], f32, tag="wtb")
                    nc.vector.tensor_copy(wtb[:], pst2[: ES + 1, :])
                    bdst = slice(c * CH + blk * 128, c * CH + (blk + 1) * 128)
                    nc.sync.dma_start(out=g0d[0:1, bdst], in_=wtb[0:1, :])
                    nc.sync.dma_start(out=g1d[0:1, bdst], in_=wtb[1:2, :])
                    nc.sync.dma_start(out=wed[0:1, bdst], in_=wtb[ES : ES + 1, :])

        # ---------------- phase 1+2: routed expert, H halves -----------------
        for hf in range(2):
            with ExitStack() as rx:
                wp = rx.enter_context(tc.tile_pool(name=f"wr{hf}", bufs=1))
                xp = rx.enter_context(tc.tile_pool(name=f"xr{hf}", bufs=2))
                hp = rx.enter_context(tc.tile_pool(name=f"hr{hf}", bufs=1))
                op_ = rx.enter_context(tc.tile_pool(name=f"or{hf}", bufs=2))
                bp = rx.enter_context(tc.tile_pool(name=f"br{hf}", bufs=2))
                pp1 = rx.enter_context(tc.tile_pool(name=f"p1r{hf}", bufs=2, space="PSUM"))
                pp2 = rx.enter_context(tc.tile_pool(name=f"p2r{hf}", bufs=2, space="PSUM"))

                w1t = []
                for k in range(8):
                    t = wp.tile([128, HH], f32r, tag=f"w1t{k}")
                    nc.sync.dma_start(
                        out=t[:], in_=w1e[k * 128 : (k + 1) * 128, hf * HH : (hf + 1) * HH]
                    )
                    w1t.append(t)
                w2t = []
                for kh in range(HH // 128):
                    t = wp.tile([128, O], f32r, tag=f"w2t{kh}")
                    nc.sync.dma_start(
                        out=t[:],
                        in_=w2e[hf * HH + kh * 128 : hf * HH + (kh + 1) * 128, :],
                    )
                    w2t.append(t)
                b1_sb = wp.tile([128, HH // 128], f32, tag="b1_sb")
                for ht in range(HH // 128):
                    _bias_col(
                        nc,
                        b1_sb[:, ht : ht + 1],
                        b1[hf * HH + ht * 128 : hf * HH + (ht + 1) * 128],
                    )
                ones_r = wp.tile([1, 128], f32, tag="ones_r")
                nc.vector.memset(ones_r[:], 1.0)
                b2_sb = wp.tile([128, OP], f32, tag="b2_sb")
                if hf == 0:
                    for o in range(OP):
                        _bias_col(nc, b2_sb[:, o : o + 1], b2[o * 128 : (o + 1) * 128])

                for c in range(nch):
                    csl = slice(c * CH, (c + 1) * CH)
                    xc = []
                    for k in range(8):
                        t = xp.tile([128, CH], f32r, tag=f"x{k}")
                        nc.sync.dma_start(
                            out=t[:], in_=xTr[k * 128 : (k + 1) * 128, csl]
                        )
                        xc.append(t)
                    wst = bp.tile([1, CH], f32, tag="wst")
                    nc.sync.dma_start(out=wst[:], in_=wed[0:1, csl])
                    wb = _bcast_row(nc, pp2, bp, ones_r, wst[:], CH, "wb")

                    hts = []
                    for ht in range(HH // 128):
                        ps = pp1.tile([128, CH], f32, tag="ps1")
                        for k in range(8):
                            nc.tensor.matmul(
                                ps[:],
                                lhsT=w1t[k][:, ht * 128 : (ht + 1) * 128],
                                rhs=xc[k][:],
                                start=(k == 0),
                                stop=(k == 7),
                            )
                        hsb = hp.tile([128, CH], f32r, tag=f"h{ht}")
                        nc.scalar.activation(
                            hsb[:], ps[:], Relu, bias=b1_sb[:, ht : ht + 1]
                        )
                        hts.append(hsb)

                    for o in range(OP):
                        ps2 = pp2.tile([128, CH], f32, tag="ps2")
                        for kh in range(HH // 128):
                            nc.tensor.matmul(
                                ps2[:],
                                lhsT=w2t[kh][:, o * 128 : (o + 1) * 128],
                                rhs=hts[kh][:],
                                start=(kh == 0),
                                stop=(kh == HH // 128 - 1),
                            )
                        ot = op_.tile([128, CH], f32, tag="ot")
                        if hf == 0:
                            nc.vector.tensor_scalar_add(ot[:], ps2[:], b2_sb[:, o : o + 1])
                            nc.vector.tensor_mul(ot[:], ot[:], wb[:])
                            nc.sync.dma_start(
                                out=acc[o * 128 : (o + 1) * 128, csl], in_=ot[:]
                            )
                        else:
                            nc.vector.tensor_mul(ot[:], ps2[:], wb[:])
                            nc.gpsimd.dma_start(
                                out=acc[o * 128 : (o + 1) * 128, csl],
                                in_=ot[:],
                                accum_op=mybir.AluOpType.add,
                            )

        # ---------------- phase 3: shared experts (H-sliced) -----------------
        with ExitStack() as sx:
            wp = sx.enter_context(tc.tile_pool(name="ws", bufs=1))
            xp = sx.enter_context(tc.tile_pool(name="xs", bufs=2))
            hp = sx.enter_context(tc.tile_pool(name="hs", bufs=1))
            op_ = sx.enter_context(tc.tile_pool(name="os", bufs=4))
            bp = sx.enter_context(tc.tile_pool(name="bs", bufs=2))
            pp1 = sx.enter_context(tc.tile_pool(name="p1s", bufs=2, space="PSUM"))
            pp2 = sx.enter_context(tc.tile_pool(name="p2s", bufs=2, space="PSUM"))

            w1st, w2st = {}, {}
            for s in range(S):
                for k in range(8):
                    t = wp.tile([128, HS], f32r, tag=f"w1s{s}_{k}")
                    nc.sync.dma_start(out=t[:], in_=w1s[s, k * 128 : (k + 1) * 128, :])
                    w1st[s, k] = t
                for kh in range(HS // 128):
                    t = wp.tile([128, O], f32r, tag=f"w2s{s}_{kh}")
                    nc.sync.dma_start(
                        out=t[:], in_=w2s[s, kh * 128 : (kh + 1) * 128, :]
                    )
                    w2st[s, kh] = t
            bs1_sb = wp.tile([128, S * (HS // 128)], f32, tag="bs1_sb")
            for s in range(S):
                for ht in range(HS // 128):
                    _bias_col(
                        nc,
                        bs1_sb[:, s * (HS // 128) + ht : s * (HS // 128) + ht + 1],
                        bs1[s, ht * 128 : (ht + 1) * 128],
                    )
            ones_s = wp.tile([1, 128], f32, tag="ones_s")
            nc.vector.memset(ones_s[:], 1.0)
            bs2_sb = wp.tile([128, S * OP], f32, tag="bs2_sb")
            for s in range(S):
                for o in range(OP):
                    _bias_col(
                        nc,
                        bs2_sb[:, s * OP + o : s * OP + o + 1],
                        bs2[s, o * 128 : (o + 1) * 128],
                    )

            for c in range(nch):
                csl = slice(c * CH, (c + 1) * CH)
                xc = []
                for k in range(8):
                    t = xp.tile([128, CH], f32r, tag=f"xs{k}")
                    nc.sync.dma_start(out=t[:], in_=xTr[k * 128 : (k + 1) * 128, csl])
                    xc.append(t)
                gb = []
                for s, gsrc in ((0, g0d), (1, g1d)):
                    st = bp.tile([1, CH], f32, tag=f"gst{s}")
                    nc.sync.dma_start(out=st[:], in_=gsrc[0:1, csl])
                    gb.append(_bcast_row(nc, pp2, bp, ones_s, st[:], CH, f"gb{s}"))

                hts = {}
                for s in range(S):
                    for ht in range(HS // 128):
                        ps = pp1.tile([128, CH], f32, tag="ps1s")
                        for k in range(8):
                            nc.tensor.matmul(
                                ps[:],
                                lhsT=w1st[s, k][:, ht * 128 : (ht + 1) * 128],
                                rhs=xc[k][:],
                                start=(k == 0),
                                stop=(k == 7),
                            )
                        hsb = hp.tile([128, CH], f32r, tag=f"hs{s}_{ht}")
                        nc.scalar.activation(
                            hsb[:],
                            ps[:],
                            Relu,
                            bias=bs1_sb[:, s * (HS // 128) + ht : s * (HS // 128) + ht + 1],
                        )
                        hts[s, ht] = hsb

                for o in range(OP):
                    acc_t = op_.tile([128, CH], f32, tag="acct")
                    for s in range(S):
                        ps2 = pp2.tile([128, CH], f32, tag="ps2s")
                        for kh in range(HS // 128):
                            nc.tensor.matmul(
                                ps2[:],
                                lhsT=w2st[s, kh][:, o * 128 : (o + 1) * 128],
                                rhs=hts[s, kh][:],
                                start=(kh == 0),
                                stop=(kh == HS // 128 - 1),
                            )
                        tmp = op_.tile([128, CH], f32, tag="tmps")
                        nc.vector.tensor_scalar_add(
                            tmp[:], ps2[:], bs2_sb[:, s * OP + o : s * OP + o + 1]
                        )
                        if s == 0:
                            nc.vector.tensor_mul(acc_t[:], tmp[:], gb[s][:])
                        else:
                            nc.vector.tensor_mul(tmp[:], tmp[:], gb[s][:])
                            nc.vector.tensor_add(acc_t[:], acc_t[:], tmp[:])
                    nc.gpsimd.dma_start(
                        out=acc[o * 128 : (o + 1) * 128, csl],
                        in_=acc_t[:],
                        accum_op=mybir.AluOpType.add,
                    )

        # ---------------- phase 4: combine across cores ----------------------
        nc.gpsimd.collective_compute(
            "ReduceScatter",
            mybir.AluOpType.add,
            replica_groups=[list(range(NC))],
            ins=[acc[:]],
            outs=[rs[:]],
        )
        nc.sync.dma_start(out=y[:], in_=rs[:])

    _split_multi_waits(nc)
    return nc


# ---------------------------------------------------------------- host side
_cache = {}


def _get_nc(nbatch):
    if nbatch not in _cache:
        _cache[nbatch] = build(nbatch)
    return _cache[nbatch]


def _make_in_maps(x, W1, b1, W2, b2, Ws1, bs1, Ws2, bs2, Wg, bg):
    x = np.asarray(x, np.float32)
    xT = np.ascontiguousarray(x.T)
    W1 = np.asarray(W1, np.float32)
    W2 = np.asarray(W2, np.float32)
    Ws1 = np.asarray(Ws1, np.float32)
    Ws2 = np.asarray(Ws2, np.float32)
    Wg = np.asarray(Wg, np.float32)
    bg = np.asarray(bg, np.float32)
    b1 = np.asarray(b1, np.float32)
    b2 = np.asarray(b2, np.float32)
    bs1 = np.asarray(bs1, np.float32)
    bs2 = np.asarray(bs2, np.float32)

    in_maps = []
    for c in range(NC):
        sel = np.zeros((1, E), np.float32)
        sel[0, c] = 1.0
        in_maps.append(
            {
                "xTf": xT,
                "xTr": xT,
                "w1e": np.ascontiguousarray(W1[c]),
                "w2e": np.ascontiguousarray(W2[c]),
                "w1s": np.ascontiguousarray(Ws1[:, :, c * HS : (c + 1) * HS]),
                "w2s": np.ascontiguousarray(Ws2[:, c * HS : (c + 1) * HS, :]),
                "wg": Wg,
                "bg": bg.reshape(ES, 1),
                "b1": np.ascontiguousarray(b1[c]),
                "b2": np.ascontiguousarray(b2[c]),
                "bs1": np.ascontiguousarray(bs1[:, c * HS : (c + 1) * HS]),
                "bs2": bs2 / float(NC),
                "sel": sel,
            }
        )
    return in_maps


_runner_cache = {}


def _get_runner(nbatch):
    """Compile (once) a non-donating SPMD runner for the built Bass module.
    Returns (fn, in_names, out_names, zero_outs, sharding)."""
    if nbatch in _runner_cache:
        return _runner_cache[nbatch]

    import jax
    from jax.experimental.shard_map import shard_map
    from jax.sharding import Mesh, NamedSharding, PartitionSpec

    from concourse import bass2jax

    nc = _get_nc(nbatch)
    partition_name = nc.partition_id_tensor.name if nc.partition_id_tensor else None
    in_names, out_names, out_avals, zero_outs = [], [], [], []
    for alloc in nc.m.functions[0].allocations:
        if not isinstance(alloc, mybir.MemoryLocationSet):
            continue
        name = alloc.memorylocations[0].name
        if alloc.kind == "ExternalInput":
            if name != partition_name:
                in_names.append(name)
        elif alloc.kind == "ExternalOutput":
            shape = tuple(alloc.tensor_shape)
            dt_ = mybir.dt.np(alloc.dtype)
            out_names.append(name)
            out_avals.append(jax.core.ShapedArray(shape, dt_))
            zero_outs.append(np.zeros(shape, dt_))
    n_params = len(in_names)
    bind_names = list(in_names) + list(out_names)
    if partition_name is not None:
        bind_names.append(partition_name)

    def _body(*args):
        operands = list(args)
        if partition_name is not None:
            operands.append(bass2jax.partition_id_tensor())
        outs = bass2jax._bass_exec_p.bind(
            *operands,
            out_avals=tuple(out_avals),
            in_names=tuple(bind_names),
            out_names=tuple(out_names),
            lowering_input_output_aliases=(),
            sim_require_finite=True,
            sim_require_nnan=True,
            nc=nc,
        )
        return tuple(outs)

    devices = jax.devices()[:NC]
    mesh = Mesh(np.asarray(devices), ("core",))
    nin = n_params + len(out_names)
    fn = jax.jit(
        shard_map(
            _body,
            mesh=mesh,
            in_specs=(PartitionSpec("core"),) * nin,
            out_specs=(PartitionSpec("core"),) * len(out_names),
            check_rep=False,
        ),
        keep_unused=True,
    )
    sh = NamedSharding(mesh, PartitionSpec("core"))
    ret = (fn, in_names, out_names, zero_outs, sh)
    _runner_cache[nbatch] = ret
    return ret


def _stage_and_run(inputs):
    """Returns (device output arrays tuple, fn, staged args)."""
    import jax

    nbatch = np.asarray(inputs["x"]).shape[0]
    in_maps = _make_in_maps(**{k: v for k, v in inputs.items() if k != "k"})
    fn, in_names, out_names, zero_outs, sh = _get_runner(nbatch)
    concat_in = [
        np.concatenate([np.asarray(in_maps[c][n]) for c in range(NC)], axis=0)
        for n in in_names
    ]
    concat_zeros = [
        np.zeros((NC * z.shape[0], *z.shape[1:]), z.dtype) for z in zero_outs
    ]
    args = [jax.device_put(a, sh) for a in concat_in + concat_zeros]
    jax.block_until_ready(args)
    out_arrs = fn(*args)
    jax.block_until_ready(out_arrs)
    return out_arrs, fn, args, out_names


def _assemble(out_arrs, out_names, nbatch):
    yc = np.asarray(out_arrs[out_names.index("y")])  # [NC * O/NC, nbatch]
    return np.ascontiguousarray(yc.T)


def kernel(x, W1, b1, W2, b2, Ws1, bs1, Ws2, bs2, Wg, bg, k):
    assert int(k) == TOPK
    inputs = dict(x=x, W1=W1, b1=b1, W2=W2, b2=b2, Ws1=Ws1, bs1=bs1,
                  Ws2=Ws2, bs2=bs2, Wg=Wg, bg=bg, k=k)
    out_arrs, _fn, _args, out_names = _stage_and_run(inputs)
    return _assemble(out_arrs, out_names, np.asarray(x).shape[0])


def bench(inputs, iters=8):
    """Run once for output, then time repeat executions with device-resident
    inputs. Returns (output, min wall ns per run)."""
    import time

    import jax

    out_arrs, fn, args, out_names = _stage_and_run(inputs)
    times = []
    for _ in range(iters):
        t0 = time.perf_counter()
        jax.block_until_ready(fn(*args))
        times.append(time.perf_counter() - t0)
    times.sort()
    print(f"bench times (s): min={times[0]:.4f} med={times[len(times)//2]:.4f} max={times[-1]:.4f}", flush=True)
    result = _assemble(out_arrs, out_names, np.asarray(inputs["x"]).shape[0])
    return result, times[0] * 1e9


# revision 17
# speedup vs baseline: 1.5303x; 1.0204x over previous
"""Trainium2 Bass kernel for nn_MoELayer (dense MoE with top-k routing).

Strategy (8 NeuronCores, SPMD):
  - Expert parallelism for the E=8 routed experts: core c owns expert c's
    MLP weights and computes w_c[token] * MLP_c(x) for ALL tokens, where
    w_c is the token's softmax gate score masked to its top-k experts
    (zero if expert c not selected).
  - Shared experts are split along the hidden dimension H: core c computes
    the H-slice [c*512,(c+1)*512) of both shared experts for all tokens,
    scaled by the shared gate scores.
  - Gate scores + softmax + top-k masking are computed on-device (fp32).
  - All per-core partial outputs live in a [O, B] feature-major DRAM
    accumulator; a ReduceScatter(add) combines them across cores, each
    core returning its [O/8, B] slice. Host concatenates + transposes.
  - Matmuls run as float32r (FP32 inputs truncated to FP22 in the PE
    array) at full 78.6 TF/s/core throughput; gate matmuls run true fp32.
"""

from contextlib import ExitStack

import numpy as np

import concourse.bass as bass
import concourse.mybir as mybir
from concourse.tile import TileContext
from concourse.masks import make_identity

# ---------------------------------------------------------------- dims
B, D, H, O = 8192, 1024, 4096, 1024
E, S = 8, 2
ES = E + S            # gate columns
NC = 8                # cores
TOPK = 2
HH = H // 2           # routed-expert H half (SBUF capacity)
HS = H // NC          # shared-expert H slice per core
CH = 512              # token chunk (matmul moving dim)
OP = O // 128         # output 128-row tiles

f32 = mybir.dt.float32
f32r = mybir.dt.float32r

# ------------------------------------------------- walrus sync-wait workaround
# This walrus build rejects any instruction carrying more than one semaphore
# wait ("Too many sync wait commands" in setupSyncWait). Tile's semaphore
# pass freely attaches several waits to one instruction. Post-process the
# serialized BIR: hoist all-but-one wait of each instruction onto standalone
# same-engine NoOps inserted immediately before it (same-engine program order
# preserves semantics exactly).
import json as _json


def _split_multi_waits(nc):
    d = _json.loads(mybir.module_to_json_string(nc.m))
    nsplit = 0
    for fn in d["functions"]:
        for bb in fn["blocks"]:
            out = []
            for inst in bb["instructions"]:
                si = inst.get("sync_info")
                waits = (si or {}).get("on_wait") or []
                if len(waits) > 1:
                    for j, w in enumerate(waits[:-1]):
                        nop = {
                            "engine": inst["engine"],
                            "ins": [],
                            "outs": [],
                            "name": f"{inst['name']}-w{j}",
                            "opcode": "NoOp",
                            "sync_info": {"on_wait": [w], "on_update": []},
                        }
                        if "debug" in inst:
                            nop["debug"] = inst["debug"]
                        out.append(nop)
                        nsplit += 1
                    si["on_wait"] = [waits[-1]]
                out.append(inst)
            bb["instructions"] = out
    nc.m = mybir.module_from_json_string(_json.dumps(d))
    return nsplit


# ---------------------------------------------------------------- builder
def _bias_col(nc, dst, src_1d):
    """DMA a length-128 1-D DRAM slice into a [128, 1] SBUF column."""
    nc.sync.dma_start(out=dst, in_=src_1d.rearrange("(p o) -> p o", o=1))



def _bcast_row(nc, psum_pool, out_pool, ones_col, row_ap, n, tag):
    """Broadcast a [1, n] SBUF row to a [128, n] tile: ones[1,128].T @ row."""
    ps = psum_pool.tile([128, n], f32, tag=tag + "_ps")
    nc.tensor.matmul(ps[:], lhsT=ones_col[:], rhs=row_ap)
    t = out_pool.tile([128, n], f32, tag=tag)
    nc.vector.tensor_copy(t[:], ps[:])
    return t


def build(nbatch: int) -> bass.Bass:
    assert nbatch % CH == 0
    nch = nbatch // CH

    nc = bass.Bass()
    xTf = nc.declare_dram_parameter("xTf", [D, nbatch], f32, isOutput=False)
    xTr = nc.declare_dram_parameter("xTr", [D, nbatch], f32r, isOutput=False)
    w1e = nc.declare_dram_parameter("w1e", [D, H], f32r, isOutput=False)
    w2e = nc.declare_dram_parameter("w2e", [H, O], f32r, isOutput=False)
    w1s = nc.declare_dram_parameter("w1s", [S, D, HS], f32r, isOutput=False)
    w2s = nc.declare_dram_parameter("w2s", [S, HS, O], f32r, isOutput=False)
    wg = nc.declare_dram_parameter("wg", [D, ES], f32, isOutput=False)
    bg = nc.declare_dram_parameter("bg", [ES, 1], f32, isOutput=False)
    b1 = nc.declare_dram_parameter("b1", [H], f32, isOutput=False)
    b2 = nc.declare_dram_parameter("b2", [O], f32, isOutput=False)
    bs1 = nc.declare_dram_parameter("bs1", [S, HS], f32, isOutput=False)
    bs2 = nc.declare_dram_parameter("bs2", [S, O], f32, isOutput=False)  # /NC on host
    sel = nc.declare_dram_parameter("sel", [1, E], f32, isOutput=False)
    y = nc.declare_dram_parameter("y", [nbatch // NC, O], f32, isOutput=True)

    acc = nc.dram_tensor("acc", [nbatch, O], f32)
    rs = nc.dram_tensor("rs", [nbatch // NC, O], f32)
    wtokd = nc.dram_tensor("wtokd", [nbatch, 3], f32)  # g0, g1, w_e per token

    Relu = mybir.ActivationFunctionType.Relu
    Ident = mybir.ActivationFunctionType.Identity
    Exp = mybir.ActivationFunctionType.Exp
    AX = mybir.AxisListType.X

    with TileContext(nc) as tc:
        # ---------------- phase 0: gate scores, softmax, top-k mask ----------
        with ExitStack() as gx:
            gconst = gx.enter_context(tc.tile_pool(name="gconst", bufs=1))
            gp = gx.enter_context(tc.tile_pool(name="gp", bufs=3))
            gxp = gx.enter_context(tc.tile_pool(name="gxp", bufs=3))
            gps = gx.enter_context(tc.tile_pool(name="gps", bufs=2, space="PSUM"))
            gps2 = gx.enter_context(tc.tile_pool(name="gps2", bufs=2, space="PSUM"))

            ident = gconst.tile([128, 128], f32, tag="ident")
            make_identity(nc, ident)
            wg_sb = gconst.tile([128, 8 * ES], f32, tag="wg_sb")
            for k in range(8):
                nc.sync.dma_start(
                    out=wg_sb[:, k * ES : (k + 1) * ES],
                    in_=wg[k * 128 : (k + 1) * 128, :],
                )
            bg_sb = gconst.tile([ES, 1], f32, tag="bg_sb")
            nc.sync.dma_start(out=bg_sb[:], in_=bg[:])
            sel_st = gconst.tile([1, E], f32, tag="sel_st")
            nc.sync.dma_start(out=sel_st[:], in_=sel[:])
            ones_g = gconst.tile([1, 128], f32, tag="ones_g")
            nc.vector.memset(ones_g[:], 1.0)
            selb = _bcast_row(nc, gps2, gconst, ones_g, sel_st[:], E, "selb")

            for c in range(nch):
                csl = slice(c * CH, (c + 1) * CH)
                xc = []
                for k in range(8):
                    t = gxp.tile([128, CH], f32, tag=f"gx{k}")
                    nc.sync.dma_start(
                        out=t[:], in_=xTf[k * 128 : (k + 1) * 128, csl]
                    )
                    xc.append(t)
                psg = gps.tile([ES, CH], f32, tag="psg")
                for k in range(8):
                    nc.tensor.matmul(
                        psg[:],
                        lhsT=wg_sb[:, k * ES : (k + 1) * ES],
                        rhs=xc[k][:],
                        start=(k == 0),
                        stop=(k == 7),
                    )
                gts = gp.tile([ES, CH], f32, tag="gts")
                nc.scalar.activation(gts[:], psg[:], Ident, bias=bg_sb[:])

                for blk in range(CH // 128):
                    bsl = slice(blk * 128, (blk + 1) * 128)
                    pst = gps2.tile([128, 128], f32, tag="pst")
                    # [ES, 128] -> [128, ES]
                    nc.tensor.matmul(
                        pst[:, :ES],
                        lhsT=gts[:, bsl],
                        rhs=ident[:ES, :ES],
                        is_transpose=True,
                    )
                    gtm = gp.tile([128, ES], f32, tag="gtm")
                    nc.vector.tensor_copy(gtm[:], pst[:, :ES])
                    mx = gp.tile([128, 1], f32, tag="mx")
                    nc.vector.reduce_max(mx[:], gtm[:], axis=AX)
                    nmx = gp.tile([128, 1], f32, tag="nmx")
                    nc.vector.tensor_scalar_mul(nmx[:], mx[:], -1.0)
                    ex = gp.tile([128, ES], f32, tag="ex")
                    nc.scalar.activation(ex[:], gtm[:], Exp, bias=nmx[:])
                    sm = gp.tile([128, 1], f32, tag="sm")
                    nc.vector.reduce_sum(sm[:], ex[:], axis=AX)
                    rc = gp.tile([128, 1], f32, tag="rc")
                    nc.vector.reciprocal(rc[:], sm[:])
                    pr = gp.tile([128, ES], f32, tag="pr")
                    nc.vector.tensor_scalar_mul(pr[:], ex[:], rc[:])
                    # top-k mask over routed columns
                    m8 = gp.tile([128, 8], f32, tag="m8")
                    nc.vector.max(m8[:], pr[:, S:])
                    nc.vector.memset(m8[:, TOPK:], -1.0)
                    rep = gp.tile([128, 8], f32, tag="rep")
                    nc.vector.match_replace(
                        rep[:], in_to_replace=m8[:], in_values=pr[:, S:], imm_value=0.0
                    )
                    wr = gp.tile([128, ES + 1], f32, tag="wr")
                    nc.vector.tensor_copy(wr[:, :S], pr[:, :S])
                    nc.vector.tensor_sub(wr[:, S : ES], pr[:, S:], rep[:])
                    # this core's expert gate: dot(masked routed, one-hot)
                    seld = gp.tile([128, E], f32, tag="seld")
                    nc.vector.tensor_mul(seld[:], wr[:, S:ES], selb[:])
                    nc.vector.reduce_sum(wr[:, ES : ES + 1], seld[:], axis=AX)
                    bdst = slice(c * CH + blk * 128, c * CH + (blk + 1) * 128)
                    nc.sync.dma_start(out=wtokd[bdst, 0:2], in_=wr[:, :S])
                    nc.sync.dma_start(out=wtokd[bdst, 2:3], in_=wr[:, ES : ES + 1])

        # ---------------- phase 1+2: routed expert, H halves -----------------
        for hf in range(2):
            with ExitStack() as rx:
                wp = rx.enter_context(tc.tile_pool(name=f"wr{hf}", bufs=1))
                xp = rx.enter_context(tc.tile_pool(name=f"xr{hf}", bufs=2))
                hp = rx.enter_context(tc.tile_pool(name=f"hr{hf}", bufs=1))
                op_ = rx.enter_context(tc.tile_pool(name=f"or{hf}", bufs=2))
                bp = rx.enter_context(tc.tile_pool(name=f"br{hf}", bufs=2))
                pp1 = rx.enter_context(tc.tile_pool(name=f"p1r{hf}", bufs=2, space="PSUM"))
                pp2 = rx.enter_context(tc.tile_pool(name=f"p2r{hf}", bufs=2, space="PSUM"))

                w1t = []
                for k in range(8):
                    t = wp.tile([128, HH], f32r, tag=f"w1t{k}")
                    nc.sync.dma_start(
                        out=t[:], in_=w1e[k * 128 : (k + 1) * 128, hf * HH : (hf + 1) * HH]
                    )
                    w1t.append(t)
                w2t = []
                for kh in range(HH // 128):
                    t = wp.tile([128, O], f32r, tag=f"w2t{kh}")
                    nc.sync.dma_start(
                        out=t[:],
                        in_=w2e[hf * HH + kh * 128 : hf * HH + (kh + 1) * 128, :],
                    )
                    w2t.append(t)
                b1_sb = wp.tile([128, HH // 128], f32, tag="b1_sb")
                for ht in range(HH // 128):
                    _bias_col(
                        nc,
                        b1_sb[:, ht : ht + 1],
                        b1[hf * HH + ht * 128 : hf * HH + (ht + 1) * 128],
                    )
                ones_r = wp.tile([1, 128], f32, tag="ones_r")
                nc.vector.memset(ones_r[:], 1.0)
                # b2 broadcast across partitions, token-major: [128, O]
                b2tm = wp.tile([128, O], f32, tag="b2tm")
                if hf == 0:
                    b2row = wp.tile([1, O], f32, tag="b2row")
                    nc.sync.dma_start(
                        out=b2row[:], in_=b2.rearrange("(a b) -> a b", a=1)
                    )
                    for o2 in range(O // CH):
                        osl = slice(o2 * CH, (o2 + 1) * CH)
                        bps = pp2.tile([128, CH], f32, tag="b2ps")
                        nc.tensor.matmul(bps[:], lhsT=ones_r[:], rhs=b2row[:, osl])
                        nc.vector.tensor_copy(b2tm[:, osl], bps[:])

                for c in range(nch):
                    csl = slice(c * CH, (c + 1) * CH)
                    xc = []
                    for k in range(8):
                        t = xp.tile([128, CH], f32r, tag=f"x{k}")
                        nc.sync.dma_start(
                            out=t[:], in_=xTr[k * 128 : (k + 1) * 128, csl]
                        )
                        xc.append(t)
                    wts = []
                    for t in range(CH // 128):
                        wt = bp.tile([128, 3], f32, tag=f"wt{t}")
                        nc.sync.dma_start(
                            out=wt[:],
                            in_=wtokd[c * CH + t * 128 : c * CH + (t + 1) * 128, :],
                        )
                        wts.append(wt)

                    hts = []
                    for ht in range(HH // 128):
                        ps = pp1.tile([128, CH], f32, tag="ps1")
                        for k in range(8):
                            nc.tensor.matmul(
                                ps[:],
                                lhsT=w1t[k][:, ht * 128 : (ht + 1) * 128],
                                rhs=xc[k][:],
                                start=(k == 0),
                                stop=(k == 7),
                            )
                        hsb = hp.tile([128, CH], f32r, tag=f"h{ht}")
                        nc.scalar.activation(
                            hsb[:], ps[:], Relu, bias=b1_sb[:, ht : ht + 1]
                        )
                        hts.append(hsb)

                    for t in range(CH // 128):
                        tsl = slice(c * CH + t * 128, c * CH + (t + 1) * 128)
                        for o2 in range(O // CH):
                            osl = slice(o2 * CH, (o2 + 1) * CH)
                            ps2 = pp2.tile([128, CH], f32, tag="ps2")
                            for kh in range(HH // 128):
                                nc.tensor.matmul(
                                    ps2[:],
                                    lhsT=hts[kh][:, t * 128 : (t + 1) * 128],
                                    rhs=w2t[kh][:, osl],
                                    start=(kh == 0),
                                    stop=(kh == HH // 128 - 1),
                                )
                            ot = op_.tile([128, CH], f32, tag="ot")
                            if hf == 0:
                                nc.vector.tensor_add(ot[:], ps2[:], b2tm[:, osl])
                                nc.vector.tensor_scalar_mul(ot[:], ot[:], wts[t][:, 2:3])
                                nc.sync.dma_start(out=acc[tsl, osl], in_=ot[:])
                            else:
                                nc.vector.tensor_scalar_mul(ot[:], ps2[:], wts[t][:, 2:3])
                                nc.gpsimd.dma_start(
                                    out=acc[tsl, osl],
                                    in_=ot[:],
                                    accum_op=mybir.AluOpType.add,
                                )

        # ---------------- phase 3: shared experts (H-sliced) -----------------
        with ExitStack() as sx:
            wp = sx.enter_context(tc.tile_pool(name="ws", bufs=1))
            xp = sx.enter_context(tc.tile_pool(name="xs", bufs=2))
            hp = sx.enter_context(tc.tile_pool(name="hs", bufs=1))
            op_ = sx.enter_context(tc.tile_pool(name="os", bufs=4))
            bp = sx.enter_context(tc.tile_pool(name="bs", bufs=2))
            pp1 = sx.enter_context(tc.tile_pool(name="p1s", bufs=2, space="PSUM"))
            pp2 = sx.enter_context(tc.tile_pool(name="p2s", bufs=2, space="PSUM"))

            w1st, w2st = {}, {}
            for s in range(S):
                for k in range(8):
                    t = wp.tile([128, HS], f32r, tag=f"w1s{s}_{k}")
                    nc.sync.dma_start(out=t[:], in_=w1s[s, k * 128 : (k + 1) * 128, :])
                    w1st[s, k] = t
                for kh in range(HS // 128):
                    t = wp.tile([128, O], f32r, tag=f"w2s{s}_{kh}")
                    nc.sync.dma_start(
                        out=t[:], in_=w2s[s, kh * 128 : (kh + 1) * 128, :]
                    )
                    w2st[s, kh] = t
            bs1_sb = wp.tile([128, S * (HS // 128)], f32, tag="bs1_sb")
            for s in range(S):
                for ht in range(HS // 128):
                    _bias_col(
                        nc,
                        bs1_sb[:, s * (HS // 128) + ht : s * (HS // 128) + ht + 1],
                        bs1[s, ht * 128 : (ht + 1) * 128],
                    )
            ones_s = wp.tile([1, 128], f32, tag="ones_s")
            nc.vector.memset(ones_s[:], 1.0)
            bs2tm = []
            for s in range(S):
                brow = wp.tile([1, O], f32, tag=f"bs2row{s}")
                nc.sync.dma_start(
                    out=brow[:], in_=bs2[s].rearrange("(a b) -> a b", a=1)
                )
                btm = wp.tile([128, O], f32, tag=f"bs2tm{s}")
                for o2 in range(O // CH):
                    osl = slice(o2 * CH, (o2 + 1) * CH)
                    bps = pp2.tile([128, CH], f32, tag="bs2ps")
                    nc.tensor.matmul(bps[:], lhsT=ones_s[:], rhs=brow[:, osl])
                    nc.vector.tensor_copy(btm[:, osl], bps[:])
                bs2tm.append(btm)

            for c in range(nch):
                csl = slice(c * CH, (c + 1) * CH)
                xc = []
                for k in range(8):
                    t = xp.tile([128, CH], f32r, tag=f"xs{k}")
                    nc.sync.dma_start(out=t[:], in_=xTr[k * 128 : (k + 1) * 128, csl])
                    xc.append(t)
                wts = []
                for t in range(CH // 128):
                    wt = bp.tile([128, 3], f32, tag=f"wts{t}")
                    nc.sync.dma_start(
                        out=wt[:],
                        in_=wtokd[c * CH + t * 128 : c * CH + (t + 1) * 128, :],
                    )
                    wts.append(wt)

                hts = {}
                for s in range(S):
                    for ht in range(HS // 128):
                        ps = pp1.tile([128, CH], f32, tag="ps1s")
                        for k in range(8):
                            nc.tensor.matmul(
                                ps[:],
                                lhsT=w1st[s, k][:, ht * 128 : (ht + 1) * 128],
                                rhs=xc[k][:],
                                start=(k == 0),
                                stop=(k == 7),
                            )
                        hsb = hp.tile([128, CH], f32r, tag=f"hs{s}_{ht}")
                        nc.scalar.activation(
                            hsb[:],
                            ps[:],
                            Relu,
                            bias=bs1_sb[:, s * (HS // 128) + ht : s * (HS // 128) + ht + 1],
                        )
                        hts[s, ht] = hsb

                for t in range(CH // 128):
                    tsl = slice(c * CH + t * 128, c * CH + (t + 1) * 128)
                    for o2 in range(O // CH):
                        osl = slice(o2 * CH, (o2 + 1) * CH)
                        acc_t = op_.tile([128, CH], f32, tag="acct")
                        for s in range(S):
                            ps2 = pp2.tile([128, CH], f32, tag="ps2s")
                            for kh in range(HS // 128):
                                nc.tensor.matmul(
                                    ps2[:],
                                    lhsT=hts[s, kh][:, t * 128 : (t + 1) * 128],
                                    rhs=w2st[s, kh][:, osl],
                                    start=(kh == 0),
                                    stop=(kh == HS // 128 - 1),
                                )
                            tmp = op_.tile([128, CH], f32, tag="tmps")
                            nc.vector.tensor_add(tmp[:], ps2[:], bs2tm[s][:, osl])
                            if s == 0:
                                nc.vector.tensor_scalar_mul(
                                    acc_t[:], tmp[:], wts[t][:, s : s + 1]
                                )
                            else:
                                nc.vector.tensor_scalar_mul(
                                    tmp[:], tmp[:], wts[t][:, s : s + 1]
                                )
                                nc.vector.tensor_add(acc_t[:], acc_t[:], tmp[:])
                        nc.gpsimd.dma_start(
                            out=acc[tsl, osl],
                            in_=acc_t[:],
                            accum_op=mybir.AluOpType.add,
                        )

        # ---------------- phase 4: combine across cores ----------------------
        ngrp = min(4, nch)
        grows = nbatch // ngrp
        rrows = grows // NC
        for g in range(ngrp):
            nc.gpsimd.collective_compute(
                "ReduceScatter",
                mybir.AluOpType.add,
                replica_groups=[list(range(NC))],
                ins=[acc[g * grows : (g + 1) * grows, :]],
                outs=[rs[g * rrows : (g + 1) * rrows, :]],
            )
            nc.sync.dma_start(
                out=y[g * rrows : (g + 1) * rrows, :],
                in_=rs[g * rrows : (g + 1) * rrows, :],
            )

    _split_multi_waits(nc)
    return nc


# ---------------------------------------------------------------- host side
_cache = {}


def _get_nc(nbatch):
    if nbatch not in _cache:
        _cache[nbatch] = build(nbatch)
    return _cache[nbatch]


def _make_in_maps(x, W1, b1, W2, b2, Ws1, bs1, Ws2, bs2, Wg, bg):
    x = np.asarray(x, np.float32)
    xT = np.ascontiguousarray(x.T)
    W1 = np.asarray(W1, np.float32)
    W2 = np.asarray(W2, np.float32)
    Ws1 = np.asarray(Ws1, np.float32)
    Ws2 = np.asarray(Ws2, np.float32)
    Wg = np.asarray(Wg, np.float32)
    bg = np.asarray(bg, np.float32)
    b1 = np.asarray(b1, np.float32)
    b2 = np.asarray(b2, np.float32)
    bs1 = np.asarray(bs1, np.float32)
    bs2 = np.asarray(bs2, np.float32)

    in_maps = []
    for c in range(NC):
        sel = np.zeros((1, E), np.float32)
        sel[0, c] = 1.0
        in_maps.append(
            {
                "xTf": xT,
                "xTr": xT,
                "w1e": np.ascontiguousarray(W1[c]),
                "w2e": np.ascontiguousarray(W2[c]),
                "w1s": np.ascontiguousarray(Ws1[:, :, c * HS : (c + 1) * HS]),
                "w2s": np.ascontiguousarray(Ws2[:, c * HS : (c + 1) * HS, :]),
                "wg": Wg,
                "bg": bg.reshape(ES, 1),
                "b1": np.ascontiguousarray(b1[c]),
                "b2": np.ascontiguousarray(b2[c]),
                "bs1": np.ascontiguousarray(bs1[:, c * HS : (c + 1) * HS]),
                "bs2": bs2 / float(NC),
                "sel": sel,
            }
        )
    return in_maps


_runner_cache = {}


def _get_runner(nbatch):
    """Compile (once) a non-donating SPMD runner for the built Bass module.
    Returns (fn, in_names, out_names, zero_outs, sharding)."""
    if nbatch in _runner_cache:
        return _runner_cache[nbatch]

    import jax
    from jax.experimental.shard_map import shard_map
    from jax.sharding import Mesh, NamedSharding, PartitionSpec

    from concourse import bass2jax

    nc = _get_nc(nbatch)
    partition_name = nc.partition_id_tensor.name if nc.partition_id_tensor else None
    in_names, out_names, out_avals, zero_outs = [], [], [], []
    for alloc in nc.m.functions[0].allocations:
        if not isinstance(alloc, mybir.MemoryLocationSet):
            continue
        name = alloc.memorylocations[0].name
        if alloc.kind == "ExternalInput":
            if name != partition_name:
                in_names.append(name)
        elif alloc.kind == "ExternalOutput":
            shape = tuple(alloc.tensor_shape)
            dt_ = mybir.dt.np(alloc.dtype)
            out_names.append(name)
            out_avals.append(jax.core.ShapedArray(shape, dt_))
            zero_outs.append(np.zeros(shape, dt_))
    n_params = len(in_names)
    bind_names = list(in_names) + list(out_names)
    if partition_name is not None:
        bind_names.append(partition_name)

    def _body(*args):
        operands = list(args)
        if partition_name is not None:
            operands.append(bass2jax.partition_id_tensor())
        outs = bass2jax._bass_exec_p.bind(
            *operands,
            out_avals=tuple(out_avals),
            in_names=tuple(bind_names),
            out_names=tuple(out_names),
            lowering_input_output_aliases=(),
            sim_require_finite=True,
            sim_require_nnan=True,
            nc=nc,
        )
        return tuple(outs)

    devices = jax.devices()[:NC]
    mesh = Mesh(np.asarray(devices), ("core",))
    nin = n_params + len(out_names)
    fn = jax.jit(
        shard_map(
            _body,
            mesh=mesh,
            in_specs=(PartitionSpec("core"),) * nin,
            out_specs=(PartitionSpec("core"),) * len(out_names),
            check_rep=False,
        ),
        keep_unused=True,
    )
    sh = NamedSharding(mesh, PartitionSpec("core"))
    ret = (fn, in_names, out_names, zero_outs, sh)
    _runner_cache[nbatch] = ret
    return ret


def _stage_and_run(inputs):
    """Returns (device output arrays tuple, fn, staged args)."""
    import jax

    nbatch = np.asarray(inputs["x"]).shape[0]
    in_maps = _make_in_maps(**{k: v for k, v in inputs.items() if k != "k"})
    fn, in_names, out_names, zero_outs, sh = _get_runner(nbatch)
    concat_in = [
        np.concatenate([np.asarray(in_maps[c][n]) for c in range(NC)], axis=0)
        for n in in_names
    ]
    concat_zeros = [
        np.zeros((NC * z.shape[0], *z.shape[1:]), z.dtype) for z in zero_outs
    ]
    args = [jax.device_put(a, sh) for a in concat_in + concat_zeros]
    jax.block_until_ready(args)
    out_arrs = fn(*args)
    jax.block_until_ready(out_arrs)
    return out_arrs, fn, args, out_names


def _assemble(out_arrs, out_names, nbatch):
    yc = np.asarray(out_arrs[out_names.index("y")])  # [NC * nbatch/NC, O]
    ys = yc.reshape(NC, nbatch // NC, O)
    ngrp = min(4, nbatch // CH)
    grows = nbatch // ngrp
    rrows = grows // NC
    out = np.empty((nbatch, O), np.float32)
    for c in range(NC):
        for g in range(ngrp):
            out[g * grows + c * rrows : g * grows + (c + 1) * rrows] = (
                ys[c, g * rrows : (g + 1) * rrows]
            )
    return out


def kernel(x, W1, b1, W2, b2, Ws1, bs1, Ws2, bs2, Wg, bg, k):
    assert int(k) == TOPK
    inputs = dict(x=x, W1=W1, b1=b1, W2=W2, b2=b2, Ws1=Ws1, bs1=bs1,
                  Ws2=Ws2, bs2=bs2, Wg=Wg, bg=bg, k=k)
    out_arrs, _fn, _args, out_names = _stage_and_run(inputs)
    return _assemble(out_arrs, out_names, np.asarray(x).shape[0])


def bench(inputs, iters=8):
    """Run once for output, then time repeat executions with device-resident
    inputs. Returns (output, min wall ns per run)."""
    import time

    import jax

    out_arrs, fn, args, out_names = _stage_and_run(inputs)
    times = []
    for _ in range(iters):
        t0 = time.perf_counter()
        jax.block_until_ready(fn(*args))
        times.append(time.perf_counter() - t0)
    times.sort()
    print(f"bench times (s): min={times[0]:.4f} med={times[len(times)//2]:.4f} max={times[-1]:.4f}", flush=True)
    result = _assemble(out_arrs, out_names, np.asarray(inputs["x"]).shape[0])
    return result, times[0] * 1e9


# revision 19
# speedup vs baseline: 1.5389x; 1.0056x over previous
"""Trainium2 Bass kernel for nn_MoELayer (dense MoE with top-k routing).

Strategy (8 NeuronCores, SPMD):
  - Expert parallelism for the E=8 routed experts: core c owns expert c's
    MLP weights and computes w_c[token] * MLP_c(x) for ALL tokens, where
    w_c is the token's softmax gate score masked to its top-k experts
    (zero if expert c not selected).
  - Shared experts are split along the hidden dimension H: core c computes
    the H-slice [c*512,(c+1)*512) of both shared experts for all tokens,
    scaled by the shared gate scores.
  - Gate scores + softmax + top-k masking are computed on-device (fp32).
  - All per-core partial outputs live in a [O, B] feature-major DRAM
    accumulator; a ReduceScatter(add) combines them across cores, each
    core returning its [O/8, B] slice. Host concatenates + transposes.
  - Matmuls run as float32r (FP32 inputs truncated to FP22 in the PE
    array) at full 78.6 TF/s/core throughput; gate matmuls run true fp32.
"""

from contextlib import ExitStack

import numpy as np

import concourse.bass as bass
import concourse.mybir as mybir
from concourse.tile import TileContext
from concourse.masks import make_identity

# ---------------------------------------------------------------- dims
B, D, H, O = 8192, 1024, 4096, 1024
E, S = 8, 2
ES = E + S            # gate columns
NC = 8                # cores
TOPK = 2
HH = H // 2           # routed-expert H half (SBUF capacity)
HS = H // NC          # shared-expert H slice per core
CH = 512              # token chunk (matmul moving dim)
OP = O // 128         # output 128-row tiles

f32 = mybir.dt.float32
f32r = mybir.dt.float32r

# ------------------------------------------------- walrus sync-wait workaround
# This walrus build rejects any instruction carrying more than one semaphore
# wait ("Too many sync wait commands" in setupSyncWait). Tile's semaphore
# pass freely attaches several waits to one instruction. Post-process the
# serialized BIR: hoist all-but-one wait of each instruction onto standalone
# same-engine NoOps inserted immediately before it (same-engine program order
# preserves semantics exactly).
import json as _json


def _split_multi_waits(nc):
    d = _json.loads(mybir.module_to_json_string(nc.m))
    nsplit = 0
    for fn in d["functions"]:
        for bb in fn["blocks"]:
            out = []
            for inst in bb["instructions"]:
                si = inst.get("sync_info")
                waits = (si or {}).get("on_wait") or []
                if len(waits) > 1:
                    for j, w in enumerate(waits[:-1]):
                        nop = {
                            "engine": inst["engine"],
                            "ins": [],
                            "outs": [],
                            "name": f"{inst['name']}-w{j}",
                            "opcode": "NoOp",
                            "sync_info": {"on_wait": [w], "on_update": []},
                        }
                        if "debug" in inst:
                            nop["debug"] = inst["debug"]
                        out.append(nop)
                        nsplit += 1
                    si["on_wait"] = [waits[-1]]
                out.append(inst)
            bb["instructions"] = out
    nc.m = mybir.module_from_json_string(_json.dumps(d))
    return nsplit


# ---------------------------------------------------------------- builder
def _bias_col(nc, dst, src_1d):
    """DMA a length-128 1-D DRAM slice into a [128, 1] SBUF column."""
    nc.sync.dma_start(out=dst, in_=src_1d.rearrange("(p o) -> p o", o=1))



def _bcast_row(nc, psum_pool, out_pool, ones_col, row_ap, n, tag):
    """Broadcast a [1, n] SBUF row to a [128, n] tile: ones[1,128].T @ row."""
    ps = psum_pool.tile([128, n], f32, tag=tag + "_ps")
    nc.tensor.matmul(ps[:], lhsT=ones_col[:], rhs=row_ap)
    t = out_pool.tile([128, n], f32, tag=tag)
    nc.vector.tensor_copy(t[:], ps[:])
    return t


def build(nbatch: int) -> bass.Bass:
    assert nbatch % CH == 0
    nch = nbatch // CH

    nc = bass.Bass()
    xTf = nc.declare_dram_parameter("xTf", [D, nbatch], f32, isOutput=False)
    xTr = nc.declare_dram_parameter("xTr", [D, nbatch], f32r, isOutput=False)
    w1e = nc.declare_dram_parameter("w1e", [D, H], f32r, isOutput=False)
    w2e = nc.declare_dram_parameter("w2e", [H, O], f32r, isOutput=False)
    w1s = nc.declare_dram_parameter("w1s", [S, D, HS], f32r, isOutput=False)
    w2s = nc.declare_dram_parameter("w2s", [S, HS, O], f32r, isOutput=False)
    wg = nc.declare_dram_parameter("wg", [D, ES], f32, isOutput=False)
    bg = nc.declare_dram_parameter("bg", [ES, 1], f32, isOutput=False)
    b1 = nc.declare_dram_parameter("b1", [H], f32, isOutput=False)
    b2 = nc.declare_dram_parameter("b2", [O], f32, isOutput=False)
    bs1 = nc.declare_dram_parameter("bs1", [S, HS], f32, isOutput=False)
    bs2 = nc.declare_dram_parameter("bs2", [S, O], f32, isOutput=False)  # /NC on host
    sel = nc.declare_dram_parameter("sel", [1, E], f32, isOutput=False)
    y = nc.declare_dram_parameter("y", [nbatch // NC, O], f32, isOutput=True)

    acc = nc.dram_tensor("acc", [nbatch, O], f32)
    rs = nc.dram_tensor("rs", [nbatch // NC, O], f32)
    wtokd = nc.dram_tensor("wtokd", [nbatch, 3], f32)  # g0, g1, w_e per token

    Relu = mybir.ActivationFunctionType.Relu
    Ident = mybir.ActivationFunctionType.Identity
    Exp = mybir.ActivationFunctionType.Exp
    AX = mybir.AxisListType.X

    with TileContext(nc) as tc:
        # ---------------- phase 0: gate scores, softmax, top-k mask ----------
        with ExitStack() as gx:
            gconst = gx.enter_context(tc.tile_pool(name="gconst", bufs=1))
            gp = gx.enter_context(tc.tile_pool(name="gp", bufs=3))
            gxp = gx.enter_context(tc.tile_pool(name="gxp", bufs=3))
            gps = gx.enter_context(tc.tile_pool(name="gps", bufs=2, space="PSUM"))
            gps2 = gx.enter_context(tc.tile_pool(name="gps2", bufs=2, space="PSUM"))

            ident = gconst.tile([128, 128], f32, tag="ident")
            make_identity(nc, ident)
            wg_sb = gconst.tile([128, 8 * ES], f32, tag="wg_sb")
            for k in range(8):
                nc.sync.dma_start(
                    out=wg_sb[:, k * ES : (k + 1) * ES],
                    in_=wg[k * 128 : (k + 1) * 128, :],
                )
            bg_sb = gconst.tile([ES, 1], f32, tag="bg_sb")
            nc.sync.dma_start(out=bg_sb[:], in_=bg[:])
            sel_st = gconst.tile([1, E], f32, tag="sel_st")
            nc.sync.dma_start(out=sel_st[:], in_=sel[:])
            ones_g = gconst.tile([1, 128], f32, tag="ones_g")
            nc.vector.memset(ones_g[:], 1.0)
            selb = _bcast_row(nc, gps2, gconst, ones_g, sel_st[:], E, "selb")

            for c in range(nch):
                csl = slice(c * CH, (c + 1) * CH)
                xc = []
                for k in range(8):
                    t = gxp.tile([128, CH], f32, tag=f"gx{k}")
                    nc.sync.dma_start(
                        out=t[:], in_=xTf[k * 128 : (k + 1) * 128, csl]
                    )
                    xc.append(t)
                psg = gps.tile([ES, CH], f32, tag="psg")
                for k in range(8):
                    nc.tensor.matmul(
                        psg[:],
                        lhsT=wg_sb[:, k * ES : (k + 1) * ES],
                        rhs=xc[k][:],
                        start=(k == 0),
                        stop=(k == 7),
                    )
                gts = gp.tile([ES, CH], f32, tag="gts")
                nc.scalar.activation(gts[:], psg[:], Ident, bias=bg_sb[:])

                for blk in range(CH // 128):
                    bsl = slice(blk * 128, (blk + 1) * 128)
                    pst = gps2.tile([128, 128], f32, tag="pst")
                    # [ES, 128] -> [128, ES]
                    nc.tensor.matmul(
                        pst[:, :ES],
                        lhsT=gts[:, bsl],
                        rhs=ident[:ES, :ES],
                        is_transpose=True,
                    )
                    gtm = gp.tile([128, ES], f32, tag="gtm")
                    nc.vector.tensor_copy(gtm[:], pst[:, :ES])
                    mx = gp.tile([128, 1], f32, tag="mx")
                    nc.vector.reduce_max(mx[:], gtm[:], axis=AX)
                    nmx = gp.tile([128, 1], f32, tag="nmx")
                    nc.vector.tensor_scalar_mul(nmx[:], mx[:], -1.0)
                    ex = gp.tile([128, ES], f32, tag="ex")
                    nc.scalar.activation(ex[:], gtm[:], Exp, bias=nmx[:])
                    sm = gp.tile([128, 1], f32, tag="sm")
                    nc.vector.reduce_sum(sm[:], ex[:], axis=AX)
                    rc = gp.tile([128, 1], f32, tag="rc")
                    nc.vector.reciprocal(rc[:], sm[:])
                    pr = gp.tile([128, ES], f32, tag="pr")
                    nc.vector.tensor_scalar_mul(pr[:], ex[:], rc[:])
                    # top-k mask over routed columns
                    m8 = gp.tile([128, 8], f32, tag="m8")
                    nc.vector.max(m8[:], pr[:, S:])
                    nc.vector.memset(m8[:, TOPK:], -1.0)
                    rep = gp.tile([128, 8], f32, tag="rep")
                    nc.vector.match_replace(
                        rep[:], in_to_replace=m8[:], in_values=pr[:, S:], imm_value=0.0
                    )
                    wr = gp.tile([128, ES + 1], f32, tag="wr")
                    nc.vector.tensor_copy(wr[:, :S], pr[:, :S])
                    nc.vector.tensor_sub(wr[:, S : ES], pr[:, S:], rep[:])
                    # this core's expert gate: dot(masked routed, one-hot)
                    seld = gp.tile([128, E], f32, tag="seld")
                    nc.vector.tensor_mul(seld[:], wr[:, S:ES], selb[:])
                    nc.vector.reduce_sum(wr[:, ES : ES + 1], seld[:], axis=AX)
                    bdst = slice(c * CH + blk * 128, c * CH + (blk + 1) * 128)
                    nc.sync.dma_start(out=wtokd[bdst, 0:2], in_=wr[:, :S])
                    nc.sync.dma_start(out=wtokd[bdst, 2:3], in_=wr[:, ES : ES + 1])

        # ---------------- phase 1+2: routed expert, H halves -----------------
        for hf in range(2):
            with ExitStack() as rx:
                wp = rx.enter_context(tc.tile_pool(name=f"wr{hf}", bufs=1))
                xp = rx.enter_context(tc.tile_pool(name=f"xr{hf}", bufs=2))
                hp = rx.enter_context(tc.tile_pool(name=f"hr{hf}", bufs=1))
                op_ = rx.enter_context(tc.tile_pool(name=f"or{hf}", bufs=2))
                bp = rx.enter_context(tc.tile_pool(name=f"br{hf}", bufs=2))
                pp1 = rx.enter_context(tc.tile_pool(name=f"p1r{hf}", bufs=3, space="PSUM"))
                pp2 = rx.enter_context(tc.tile_pool(name=f"p2r{hf}", bufs=2, space="PSUM"))

                w1t = []
                for k in range(8):
                    t = wp.tile([128, HH], f32r, tag=f"w1t{k}")
                    nc.sync.dma_start(
                        out=t[:], in_=w1e[k * 128 : (k + 1) * 128, hf * HH : (hf + 1) * HH]
                    )
                    w1t.append(t)
                w2t = []
                for kh in range(HH // 128):
                    t = wp.tile([128, O], f32r, tag=f"w2t{kh}")
                    nc.sync.dma_start(
                        out=t[:],
                        in_=w2e[hf * HH + kh * 128 : hf * HH + (kh + 1) * 128, :],
                    )
                    w2t.append(t)
                b1_sb = wp.tile([128, HH // 128], f32, tag="b1_sb")
                for ht in range(HH // 128):
                    _bias_col(
                        nc,
                        b1_sb[:, ht : ht + 1],
                        b1[hf * HH + ht * 128 : hf * HH + (ht + 1) * 128],
                    )
                ones_r = wp.tile([1, 128], f32, tag="ones_r")
                nc.vector.memset(ones_r[:], 1.0)
                # b2 broadcast across partitions, token-major: [128, O]
                b2tm = wp.tile([128, O], f32, tag="b2tm")
                if hf == 0:
                    b2row = wp.tile([1, O], f32, tag="b2row")
                    nc.sync.dma_start(
                        out=b2row[:], in_=b2.rearrange("(a b) -> a b", a=1)
                    )
                    for o2 in range(O // CH):
                        osl = slice(o2 * CH, (o2 + 1) * CH)
                        bps = pp2.tile([128, CH], f32, tag="b2ps")
                        nc.tensor.matmul(bps[:], lhsT=ones_r[:], rhs=b2row[:, osl])
                        nc.vector.tensor_copy(b2tm[:, osl], bps[:])

                for c in range(nch):
                    csl = slice(c * CH, (c + 1) * CH)
                    xc = []
                    for k in range(8):
                        t = xp.tile([128, CH], f32r, tag=f"x{k}")
                        nc.sync.dma_start(
                            out=t[:], in_=xTr[k * 128 : (k + 1) * 128, csl]
                        )
                        xc.append(t)
                    wts = []
                    for t in range(CH // 128):
                        wt = bp.tile([128, 3], f32, tag=f"wt{t}")
                        nc.sync.dma_start(
                            out=wt[:],
                            in_=wtokd[c * CH + t * 128 : c * CH + (t + 1) * 128, :],
                        )
                        wts.append(wt)

                    hts = []
                    for ht in range(HH // 128):
                        ps = pp1.tile([128, CH], f32, tag="ps1")
                        for k in range(8):
                            nc.tensor.matmul(
                                ps[:],
                                lhsT=w1t[k][:, ht * 128 : (ht + 1) * 128],
                                rhs=xc[k][:],
                                start=(k == 0),
                                stop=(k == 7),
                            )
                        hsb = hp.tile([128, CH], f32r, tag=f"h{ht}")
                        nc.scalar.activation(
                            hsb[:], ps[:], Relu, bias=b1_sb[:, ht : ht + 1]
                        )
                        hts.append(hsb)

                    for t in range(CH // 128):
                        tsl = slice(c * CH + t * 128, c * CH + (t + 1) * 128)
                        for o2 in range(O // CH):
                            osl = slice(o2 * CH, (o2 + 1) * CH)
                            ps2 = pp2.tile([128, CH], f32, tag="ps2")
                            for kh in range(HH // 128):
                                nc.tensor.matmul(
                                    ps2[:],
                                    lhsT=hts[kh][:, t * 128 : (t + 1) * 128],
                                    rhs=w2t[kh][:, osl],
                                    start=(kh == 0),
                                    stop=(kh == HH // 128 - 1),
                                )
                            ot = op_.tile([128, CH], f32, tag="ot")
                            if hf == 0:
                                nc.vector.tensor_add(ot[:], ps2[:], b2tm[:, osl])
                                nc.vector.tensor_scalar_mul(ot[:], ot[:], wts[t][:, 2:3])
                                nc.sync.dma_start(out=acc[tsl, osl], in_=ot[:])
                            else:
                                nc.vector.tensor_scalar_mul(ot[:], ps2[:], wts[t][:, 2:3])
                                nc.gpsimd.dma_start(
                                    out=acc[tsl, osl],
                                    in_=ot[:],
                                    accum_op=mybir.AluOpType.add,
                                )

        # ---------------- phase 3: shared experts (H-sliced) -----------------
        with ExitStack() as sx:
            wp = sx.enter_context(tc.tile_pool(name="ws", bufs=1))
            xp = sx.enter_context(tc.tile_pool(name="xs", bufs=2))
            hp = sx.enter_context(tc.tile_pool(name="hs", bufs=1))
            op_ = sx.enter_context(tc.tile_pool(name="os", bufs=4))
            bp = sx.enter_context(tc.tile_pool(name="bs", bufs=2))
            pp1 = sx.enter_context(tc.tile_pool(name="p1s", bufs=2, space="PSUM"))
            pp2 = sx.enter_context(tc.tile_pool(name="p2s", bufs=2, space="PSUM"))

            w1st, w2st = {}, {}
            for s in range(S):
                for k in range(8):
                    t = wp.tile([128, HS], f32r, tag=f"w1s{s}_{k}")
                    nc.sync.dma_start(out=t[:], in_=w1s[s, k * 128 : (k + 1) * 128, :])
                    w1st[s, k] = t
                for kh in range(HS // 128):
                    t = wp.tile([128, O], f32r, tag=f"w2s{s}_{kh}")
                    nc.sync.dma_start(
                        out=t[:], in_=w2s[s, kh * 128 : (kh + 1) * 128, :]
                    )
                    w2st[s, kh] = t
            bs1_sb = wp.tile([128, S * (HS // 128)], f32, tag="bs1_sb")
            for s in range(S):
                for ht in range(HS // 128):
                    _bias_col(
                        nc,
                        bs1_sb[:, s * (HS // 128) + ht : s * (HS // 128) + ht + 1],
                        bs1[s, ht * 128 : (ht + 1) * 128],
                    )
            ones_s = wp.tile([1, 128], f32, tag="ones_s")
            nc.vector.memset(ones_s[:], 1.0)
            bs2tm = []
            for s in range(S):
                brow = wp.tile([1, O], f32, tag=f"bs2row{s}")
                nc.sync.dma_start(
                    out=brow[:], in_=bs2[s].rearrange("(a b) -> a b", a=1)
                )
                btm = wp.tile([128, O], f32, tag=f"bs2tm{s}")
                for o2 in range(O // CH):
                    osl = slice(o2 * CH, (o2 + 1) * CH)
                    bps = pp2.tile([128, CH], f32, tag="bs2ps")
                    nc.tensor.matmul(bps[:], lhsT=ones_s[:], rhs=brow[:, osl])
                    nc.vector.tensor_copy(btm[:, osl], bps[:])
                bs2tm.append(btm)

            for c in range(nch):
                csl = slice(c * CH, (c + 1) * CH)
                xc = []
                for k in range(8):
                    t = xp.tile([128, CH], f32r, tag=f"xs{k}")
                    nc.sync.dma_start(out=t[:], in_=xTr[k * 128 : (k + 1) * 128, csl])
                    xc.append(t)
                wts = []
                for t in range(CH // 128):
                    wt = bp.tile([128, 3], f32, tag=f"wts{t}")
                    nc.sync.dma_start(
                        out=wt[:],
                        in_=wtokd[c * CH + t * 128 : c * CH + (t + 1) * 128, :],
                    )
                    wts.append(wt)

                hts = {}
                for s in range(S):
                    for ht in range(HS // 128):
                        ps = pp1.tile([128, CH], f32, tag="ps1s")
                        for k in range(8):
                            nc.tensor.matmul(
                                ps[:],
                                lhsT=w1st[s, k][:, ht * 128 : (ht + 1) * 128],
                                rhs=xc[k][:],
                                start=(k == 0),
                                stop=(k == 7),
                            )
                        hsb = hp.tile([128, CH], f32r, tag=f"hs{s}_{ht}")
                        nc.scalar.activation(
                            hsb[:],
                            ps[:],
                            Relu,
                            bias=bs1_sb[:, s * (HS // 128) + ht : s * (HS // 128) + ht + 1],
                        )
                        hts[s, ht] = hsb

                for t in range(CH // 128):
                    tsl = slice(c * CH + t * 128, c * CH + (t + 1) * 128)
                    for o2 in range(O // CH):
                        osl = slice(o2 * CH, (o2 + 1) * CH)
                        acc_t = op_.tile([128, CH], f32, tag="acct")
                        for s in range(S):
                            ps2 = pp2.tile([128, CH], f32, tag="ps2s")
                            for kh in range(HS // 128):
                                nc.tensor.matmul(
                                    ps2[:],
                                    lhsT=hts[s, kh][:, t * 128 : (t + 1) * 128],
                                    rhs=w2st[s, kh][:, osl],
                                    start=(kh == 0),
                                    stop=(kh == HS // 128 - 1),
                                )
                            tmp = op_.tile([128, CH], f32, tag="tmps")
                            nc.vector.tensor_add(tmp[:], ps2[:], bs2tm[s][:, osl])
                            if s == 0:
                                nc.vector.tensor_scalar_mul(
                                    acc_t[:], tmp[:], wts[t][:, s : s + 1]
                                )
                            else:
                                nc.vector.tensor_scalar_mul(
                                    tmp[:], tmp[:], wts[t][:, s : s + 1]
                                )
                                nc.vector.tensor_add(acc_t[:], acc_t[:], tmp[:])
                        nc.gpsimd.dma_start(
                            out=acc[tsl, osl],
                            in_=acc_t[:],
                            accum_op=mybir.AluOpType.add,
                        )

        # ---------------- phase 4: combine across cores ----------------------
        ngrp = min(4, nch)
        grows = nbatch // ngrp
        rrows = grows // NC
        for g in range(ngrp):
            nc.gpsimd.collective_compute(
                "ReduceScatter",
                mybir.AluOpType.add,
                replica_groups=[list(range(NC))],
                ins=[acc[g * grows : (g + 1) * grows, :]],
                outs=[rs[g * rrows : (g + 1) * rrows, :]],
            )
            nc.sync.dma_start(
                out=y[g * rrows : (g + 1) * rrows, :],
                in_=rs[g * rrows : (g + 1) * rrows, :],
            )

    _split_multi_waits(nc)
    return nc


# ---------------------------------------------------------------- host side
_cache = {}


def _get_nc(nbatch):
    if nbatch not in _cache:
        _cache[nbatch] = build(nbatch)
    return _cache[nbatch]


def _make_in_maps(x, W1, b1, W2, b2, Ws1, bs1, Ws2, bs2, Wg, bg):
    x = np.asarray(x, np.float32)
    xT = np.ascontiguousarray(x.T)
    W1 = np.asarray(W1, np.float32)
    W2 = np.asarray(W2, np.float32)
    Ws1 = np.asarray(Ws1, np.float32)
    Ws2 = np.asarray(Ws2, np.float32)
    Wg = np.asarray(Wg, np.float32)
    bg = np.asarray(bg, np.float32)
    b1 = np.asarray(b1, np.float32)
    b2 = np.asarray(b2, np.float32)
    bs1 = np.asarray(bs1, np.float32)
    bs2 = np.asarray(bs2, np.float32)

    in_maps = []
    for c in range(NC):
        sel = np.zeros((1, E), np.float32)
        sel[0, c] = 1.0
        in_maps.append(
            {
                "xTf": xT,
                "xTr": xT,
                "w1e": np.ascontiguousarray(W1[c]),
                "w2e": np.ascontiguousarray(W2[c]),
                "w1s": np.ascontiguousarray(Ws1[:, :, c * HS : (c + 1) * HS]),
                "w2s": np.ascontiguousarray(Ws2[:, c * HS : (c + 1) * HS, :]),
                "wg": Wg,
                "bg": bg.reshape(ES, 1),
                "b1": np.ascontiguousarray(b1[c]),
                "b2": np.ascontiguousarray(b2[c]),
                "bs1": np.ascontiguousarray(bs1[:, c * HS : (c + 1) * HS]),
                "bs2": bs2 / float(NC),
                "sel": sel,
            }
        )
    return in_maps


_runner_cache = {}


def _get_runner(nbatch):
    """Compile (once) a non-donating SPMD runner for the built Bass module.
    Returns (fn, in_names, out_names, zero_outs, sharding)."""
    if nbatch in _runner_cache:
        return _runner_cache[nbatch]

    import jax
    from jax.experimental.shard_map import shard_map
    from jax.sharding import Mesh, NamedSharding, PartitionSpec

    from concourse import bass2jax

    nc = _get_nc(nbatch)
    partition_name = nc.partition_id_tensor.name if nc.partition_id_tensor else None
    in_names, out_names, out_avals, zero_outs = [], [], [], []
    for alloc in nc.m.functions[0].allocations:
        if not isinstance(alloc, mybir.MemoryLocationSet):
            continue
        name = alloc.memorylocations[0].name
        if alloc.kind == "ExternalInput":
            if name != partition_name:
                in_names.append(name)
        elif alloc.kind == "ExternalOutput":
            shape = tuple(alloc.tensor_shape)
            dt_ = mybir.dt.np(alloc.dtype)
            out_names.append(name)
            out_avals.append(jax.core.ShapedArray(shape, dt_))
            zero_outs.append(np.zeros(shape, dt_))
    n_params = len(in_names)
    bind_names = list(in_names) + list(out_names)
    if partition_name is not None:
        bind_names.append(partition_name)

    def _body(*args):
        operands = list(args)
        if partition_name is not None:
            operands.append(bass2jax.partition_id_tensor())
        outs = bass2jax._bass_exec_p.bind(
            *operands,
            out_avals=tuple(out_avals),
            in_names=tuple(bind_names),
            out_names=tuple(out_names),
            lowering_input_output_aliases=(),
            sim_require_finite=True,
            sim_require_nnan=True,
            nc=nc,
        )
        return tuple(outs)

    devices = jax.devices()[:NC]
    mesh = Mesh(np.asarray(devices), ("core",))
    nin = n_params + len(out_names)
    fn = jax.jit(
        shard_map(
            _body,
            mesh=mesh,
            in_specs=(PartitionSpec("core"),) * nin,
            out_specs=(PartitionSpec("core"),) * len(out_names),
            check_rep=False,
        ),
        keep_unused=True,
    )
    sh = NamedSharding(mesh, PartitionSpec("core"))
    ret = (fn, in_names, out_names, zero_outs, sh)
    _runner_cache[nbatch] = ret
    return ret


def _stage_and_run(inputs):
    """Returns (device output arrays tuple, fn, staged args)."""
    import jax

    nbatch = np.asarray(inputs["x"]).shape[0]
    in_maps = _make_in_maps(**{k: v for k, v in inputs.items() if k != "k"})
    fn, in_names, out_names, zero_outs, sh = _get_runner(nbatch)
    concat_in = [
        np.concatenate([np.asarray(in_maps[c][n]) for c in range(NC)], axis=0)
        for n in in_names
    ]
    concat_zeros = [
        np.zeros((NC * z.shape[0], *z.shape[1:]), z.dtype) for z in zero_outs
    ]
    args = [jax.device_put(a, sh) for a in concat_in + concat_zeros]
    jax.block_until_ready(args)
    out_arrs = fn(*args)
    jax.block_until_ready(out_arrs)
    return out_arrs, fn, args, out_names


def _assemble(out_arrs, out_names, nbatch):
    yc = np.asarray(out_arrs[out_names.index("y")])  # [NC * nbatch/NC, O]
    ys = yc.reshape(NC, nbatch // NC, O)
    ngrp = min(4, nbatch // CH)
    grows = nbatch // ngrp
    rrows = grows // NC
    out = np.empty((nbatch, O), np.float32)
    for c in range(NC):
        for g in range(ngrp):
            out[g * grows + c * rrows : g * grows + (c + 1) * rrows] = (
                ys[c, g * rrows : (g + 1) * rrows]
            )
    return out


def kernel(x, W1, b1, W2, b2, Ws1, bs1, Ws2, bs2, Wg, bg, k):
    assert int(k) == TOPK
    inputs = dict(x=x, W1=W1, b1=b1, W2=W2, b2=b2, Ws1=Ws1, bs1=bs1,
                  Ws2=Ws2, bs2=bs2, Wg=Wg, bg=bg, k=k)
    out_arrs, _fn, _args, out_names = _stage_and_run(inputs)
    return _assemble(out_arrs, out_names, np.asarray(x).shape[0])


def bench(inputs, iters=8):
    """Run once for output, then time repeat executions with device-resident
    inputs. Returns (output, min wall ns per run)."""
    import time

    import jax

    out_arrs, fn, args, out_names = _stage_and_run(inputs)
    times = []
    for _ in range(iters):
        t0 = time.perf_counter()
        jax.block_until_ready(fn(*args))
        times.append(time.perf_counter() - t0)
    times.sort()
    print(f"bench times (s): min={times[0]:.4f} med={times[len(times)//2]:.4f} max={times[-1]:.4f}", flush=True)
    result = _assemble(out_arrs, out_names, np.asarray(inputs["x"]).shape[0])
    return result, times[0] * 1e9
